# revision 33
# baseline (speedup 1.0000x reference)
"""BitNet transformer block on 8 Trainium2 NeuronCores (Megatron tensor-parallel).

Self-contained: builds one SPMD Bass/Tile program, shards inputs on host,
runs via run_bass_kernel_spmd, gathers the output.

v3 design (vs v2): eliminate tensor-engine idle (v2 trace: PE union-busy
560us of 1045us span; ~440us of idle gaps around collectives).
  - AG1/AG2 of activations split into two 512KB column chunks (one per local
    row tile); QKV / gate-up process row-chunks as they arrive.
  - Attention scores batched: 4 query tiles per matmul (N=512) against each
    key tile. Query tiles live in qkT in chunk-slot order (evens then odds)
    so quads are contiguous.
  - Attention outputs are AllToAll'd RAW (bf16, natural [rows, feats]
    layout) in two chunks fired mid-attention; the int8-grid quantization
    for o-proj happens AFTER the A2A where each core owns all features of
    its rows, so the per-row a_scale is local. This removes v2's four
    2KB AllReduce(max) ops (12-25us each) and the 8KB ReduceScatter.
  - o-proj / x1 / rmsnorm2 / AG2 pipelined per row tile.
  - MLP: m tiles kept in bf16 so all 16 stay resident (no buffer
    starvation around the scale AllReduces); gate/up uses [P,512] PSUM
    chunks; down-proj ReduceScatter split per (oc, even/odd row tiles)
    into 8 x 1MB chunks so the exposed tail is one 1MB RS.

Numerics: quantized activations (ints in [-127,127]) and ternary weights are
exact in bf16; matmuls accumulate in fp32 PSUM, so every BitNet matmul is
exact integer arithmetic. Attention outputs and m cross the wire / live in
bf16 before their quantization: this adds <=0.25 int-step of extra rounding
on top of the inherent 0.5-step quant noise. Rounding uses the fp32
magic-constant trick (+1.5*2^23) matching jnp.round ties-to-even.
"""

import os

import numpy as np
import ml_dtypes

import concourse.bacc as bacc
import concourse.mybir as mybir
import concourse.tile as tile
from concourse.bass_utils import run_bass_kernel_spmd

F32 = mybir.dt.float32
BF16 = mybir.dt.bfloat16
AF = mybir.ActivationFunctionType
ALU = mybir.AluOpType
AX = mybir.AxisListType

NCORES = 8
B, S, D, H, MLP = 2, 1024, 2048, 16, 8192
HD = 128
R = B * S                 # 2048 rows total
RL = R // NCORES          # 256 rows per core (row shard)
OQ = D // NCORES          # 256 qkv out cols per core (2 heads)
OM = MLP // NCORES        # 1024 mlp cols per core
P = 128
KT = D // P               # 16 feature chunks
RT = R // P               # 16 row tiles
LT = RL // P              # 2 local row tiles
ST = S // P               # 8 seq tiles per batch
MT = OM // P              # 8 mlp k-chunks per core
MAGIC = 12582912.0        # 1.5 * 2**23: fp32 round-to-nearest-even magic
INV_SQRT_HD = 1.0 / float(np.sqrt(HD))

_CACHED_NC = None


def SLOT(t):
    """qkT column slot for global row tile t (evens first, then odds)."""
    return (t % 2) * 8 + t // 2


def _quant(nc, sp, src_ap, qscale_ap, out_bf_ap, tag="qtmp"):
    """out_bf = round(src * qscale) as bf16.

    fp32 +MAGIC rounds to integer (RNE); ACT subtracts MAGIC and casts to
    bf16 (small ints are exact in bf16).
    """
    F = src_ap.shape[1]
    CH = min(F, 1024)  # bound the fp32 scratch to 4KB/partition
    for c0 in range(0, F, CH):
        tmp = sp.tile([src_ap.shape[0], CH], F32, tag=tag, name=tag)
        nc.vector.tensor_scalar(
            tmp[:], src_ap[:, c0:c0 + CH], qscale_ap, MAGIC,
            op0=ALU.mult, op1=ALU.add,
        )
        nc.scalar.activation(
            out_bf_ap[:, c0:c0 + CH], tmp[:], AF.Copy, bias=-MAGIC, scale=1.0
        )


class _FixedTilePool:
    """Adapter handing out a pre-allocated tile (for _rms_quant_rows sqd)."""

    def __init__(self, t):
        self._t = t

    def tile(self, shape, dtype, tag=""):
        return self._t


def _rms_quant_rows(nc, sp, ps_dummy, src_tile, nw_tile, as_out_ap, aq_out_ap):
    """rmsnorm + abs-max + int8-grid quantize for one [128, D] row tile.

    Writes a_scale (max|h|+1e-8, h = src/rms*nw) to as_out_ap [128,1] and
    the quantized bf16 ints to aq_out_ap. Mutates src_tile in place
    (src *= nw).
    """
    sqd = ps_dummy.tile([P, D], F32, tag="sqd")
    ssq = sp.tile([P, 1], F32, tag="ssq")
    nc.scalar.activation(sqd[:], src_tile[:], AF.Square, accum_out=ssq[:])
    rms = sp.tile([P, 1], F32, tag="rms")
    nc.vector.tensor_scalar(
        rms[:], ssq[:], 1.0 / D, 1e-6, op0=ALU.mult, op1=ALU.add
    )
    nc.scalar.activation(rms[:], rms[:], AF.Sqrt)
    rinv = sp.tile([P, 1], F32, tag="rinv")
    nc.vector.reciprocal(rinv[:], rms[:])
    nc.vector.tensor_tensor(src_tile[:], src_tile[:], nw_tile[:], op=ALU.mult)
    amax = sp.tile([P, 1], F32, tag="amax")
    nc.vector.tensor_reduce(
        amax[:], src_tile[:], op=ALU.max, axis=AX.X, apply_absolute_value=True
    )
    nc.vector.tensor_scalar(
        as_out_ap, amax[:], rinv[:], 1e-8, op0=ALU.mult, op1=ALU.add
    )
    inva = sp.tile([P, 1], F32, tag="inva")
    nc.vector.reciprocal(inva[:], as_out_ap)
    qs = sp.tile([P, 1], F32, tag="qs")
    nc.vector.tensor_scalar(
        qs[:], inva[:], rinv[:], 127.0, op0=ALU.mult, op1=ALU.mult
    )
    _quant(nc, sp, src_tile[:], qs[:, 0:1], aq_out_ap)


def build_program():
    nc = bacc.Bacc(
        "TRN2",
        target_bir_lowering=False,
        debug=False,
        enable_asserts=True,
        num_devices=NCORES,
    )
    rg = [list(range(NCORES))]

    # ---------------- I/O (identical layouts to v2) ----------------
    x_rows = nc.dram_tensor("x_rows", [RL, D], F32, kind="ExternalInput").ap()
    wqkvT = nc.dram_tensor("wqkvT", [D, 3 * OQ], BF16, kind="ExternalInput").ap()
    woT = nc.dram_tensor("woT", [D, D], BF16, kind="ExternalInput").ap()
    wguT = nc.dram_tensor("wguT", [D, 2 * OM], BF16, kind="ExternalInput").ap()
    wdT = nc.dram_tensor("wdT", [OM, D], BF16, kind="ExternalInput").ap()
    norm1_w = nc.dram_tensor("norm1_w", [1, D], F32, kind="ExternalInput").ap()
    norm2_w = nc.dram_tensor("norm2_w", [1, D], F32, kind="ExternalInput").ap()
    ident_b = nc.dram_tensor("ident_b", [P, P], BF16, kind="ExternalInput").ap()
    causal_t = nc.dram_tensor("causal_t", [P, P], F32, kind="ExternalInput").ap()
    wconsts = nc.dram_tensor("wconsts", [1, 8], F32, kind="ExternalInput").ap()
    rowsel_d = nc.dram_tensor("rowsel", [1, NCORES], F32,
                              kind="ExternalInput").ap()
    out_d = nc.dram_tensor("out", [RL, D], F32, kind="ExternalOutput").ap()

    with tile.TileContext(nc) as tc, \
         tc.tile_pool(name="persist", bufs=1) as pp, \
         tc.tile_pool(name="dram", bufs=1, space="DRAM") as dp:

        # ---------------- constants ----------------
        ident = pp.tile([P, P], BF16, tag="ident")
        nc.sync.dma_start(ident[:], ident_b)
        maskT = pp.tile([P, P], F32, tag="maskT")
        nc.sync.dma_start(maskT[:], causal_t)
        wcrow = pp.tile([1, 8], F32, tag="wcrow")
        nc.sync.dma_start(wcrow[:], wconsts)
        # 0: wsq*wsk/(127^2 sqrt(HD)), 1: wsv/127, 2: wso/127,
        # 3: wsg/127, 4: wsu/127, 5: wsd/127
        cb = {}
        for slot in range(6):
            cb[slot] = pp.tile([P, 1], F32, tag=f"cb{slot}", name=f"cb{slot}")
            nc.gpsimd.partition_broadcast(
                cb[slot][:], wcrow[0:1, slot:slot + 1]
            )

        # persistent scale tiles
        as1g = pp.tile([P, RT], F32, tag="as1g")
        scv = pp.tile([P, RT], F32, tag="scv")
        asol = pp.tile([P, LT], F32, tag="asol")
        sc_ol = pp.tile([P, LT], F32, tag="sc_ol")
        qs_o = pp.tile([P, LT], F32, tag="qs_o")
        as2g = pp.tile([P, RT], F32, tag="as2g")
        sc_g = pp.tile([P, RT], F32, tag="sc_g")
        sc_u = pp.tile([P, RT], F32, tag="sc_u")
        asm = pp.tile([P, RT], F32, tag="asm")      # proc-order columns
        asmg = pp.tile([P, RT], F32, tag="asmg")
        qsm = pp.tile([P, RT], F32, tag="qsm")
        asml = pp.tile([P, LT], F32, tag="asml")
        sc_dl = pp.tile([P, LT], F32, tag="sc_dl")

        # collective DRAM buffers
        ag1c_in = [dp.tile([D, P], BF16, tag=f"ag1ci{h}", name=f"ag1ci{h}")
                   for h in range(LT)]
        ag1c_out = [dp.tile([NCORES * D, P], BF16, tag=f"ag1co{h}",
                            name=f"ag1co{h}", addr_space="Shared")
                    for h in range(LT)]
        ag1s_in = dp.tile([RL, 1], F32, tag="ag1s_in")
        ag1s_out = dp.tile([R, 1], F32, tag="ag1s_out", addr_space="Shared")
        a2a_in = [dp.tile([NCORES * P, 2 * P], BF16, tag=f"a2ai{h}",
                          name=f"a2ai{h}") for h in range(LT)]
        a2a_out = [dp.tile([NCORES * P, 2 * P], BF16, tag=f"a2ao{h}",
                           name=f"a2ao{h}") for h in range(LT)]
        x1_d = dp.tile([RL, D], F32, tag="x1_d")
        ag2c_in = [dp.tile([D, P], BF16, tag=f"ag2ci{h}", name=f"ag2ci{h}")
                   for h in range(LT)]
        ag2c_out = [dp.tile([NCORES * D, P], BF16, tag=f"ag2co{h}",
                            name=f"ag2co{h}", addr_space="Shared")
                    for h in range(LT)]
        ag2s_in = dp.tile([RL, 1], F32, tag="ag2s_in")
        ag2s_out = dp.tile([R, 1], F32, tag="ag2s_out", addr_space="Shared")

        # =========================================================
        # Mega-pool 1: phases 1,2,3 + o-proj + phase 4
        # =========================================================
        with tc.tile_pool(name="mp1", bufs=1) as m1:
            nw1 = m1.tile([P, D], F32, tag="nw1")
            cqb = m1.tile([P, R], F32, tag="cqb")
            qkT = [m1.tile([P, R], BF16, tag=f"qkT{ch}", name=f"qkT{ch}")
                   for ch in range(4)]
            vplus = [m1.tile([P, 258], BF16, tag=f"vp{t}", name=f"vp{t}")
                     for t in range(RT)]
            x1t = [m1.tile([P, D], F32, tag=f"x1t{lt}", name=f"x1t{lt}")
                   for lt in range(LT)]

            # ---- Phase 1: local rmsnorm1 + quant + transpose + chunked AG
            with tc.tile_pool(name="p2w", bufs=1) as p2m:
                wqkv_sb = [p2m.tile([P, 3 * OQ], BF16, tag=f"wqkv{k}",
                                    name=f"wqkv{k}") for k in range(KT)]
                with tc.tile_pool(name="p1sc", bufs=2) as s1, \
                     tc.tile_pool(name="ps1", bufs=1, space="PSUM") as ps1, \
                     tc.tile_pool(name="ps1t", bufs=2, space="PSUM") as ps1t:
                    # pre-emit x loads (sync); weight prefetch on scalar queue
                    xt = [s1.tile([P, D], F32, tag=f"xt{lt}", name=f"xt{lt}",
                                  bufs=1) for lt in range(LT)]
                    for lt in range(LT):
                        nc.sync.dma_start(
                            xt[lt][:], x_rows[lt * P:(lt + 1) * P, :]
                        )
                    for k in range(KT):
                        nc.scalar.dma_start(
                            wqkv_sb[k][:], wqkvT[k * P:(k + 1) * P, :]
                        )
                    nwr = s1.tile([1, D], F32, tag="nwr", bufs=1)
                    nc.sync.dma_start(nwr[:], norm1_w)
                    nc.gpsimd.partition_broadcast(nw1[:], nwr[0:1, :])
                    for lt in range(LT):
                        as_l = s1.tile([P, 1], F32, tag="as_l")
                        aq = s1.tile([P, D], BF16, tag="aq", bufs=1)
                        _rms_quant_rows(nc, s1, ps1, xt[lt], nw1,
                                        as_l[:, 0:1], aq[:])
                        nc.sync.dma_start(
                            ag1s_in[lt * P:(lt + 1) * P, :], as_l[:]
                        )
                        for kb in range(KT):
                            pst = ps1t.tile([P, P], BF16, tag="pst")
                            nc.tensor.transpose(
                                pst[:], aq[:, kb * P:(kb + 1) * P], ident[:]
                            )
                            aqs = s1.tile([P, P], BF16, tag="aqs", bufs=3)
                            nc.vector.tensor_copy(aqs[:], pst[:])
                            nc.sync.dma_start(
                                ag1c_in[lt][kb * P:(kb + 1) * P, :], aqs[:]
                            )
                        if lt == 1:
                            # scales AG between the two act chunks (its input
                            # is complete once both as_l DMAs have landed)
                            nc.gpsimd.collective_compute(
                                "AllGather", ALU.bypass, replica_groups=rg,
                                ins=[ag1s_in.opt()], outs=[ag1s_out.opt()],
                            )
                        nc.gpsimd.collective_compute(
                            "AllGather", ALU.bypass, replica_groups=rg,
                            ins=[ag1c_in[lt].opt()], outs=[ag1c_out[lt].opt()],
                        )

                # ---- Phase 2: QKV per AG chunk ----
                # qkT[ch] columns in chunk-slot order: tile t at SLOT(t)*P
                with tc.tile_pool(name="ps2qk", bufs=2, space="PSUM") as ps2qk, \
                     tc.tile_pool(name="ps2v", bufs=2, space="PSUM") as ps2v:
                    for h in range(LT):
                        a1T = [p2m.tile([P, NCORES * P], BF16,
                                        tag=f"a1T{kb}", name=f"a1T{kb}",
                                        bufs=2) for kb in range(KT)]
                        src = ag1c_out[h].rearrange(
                            "(c k p) j -> k p c j", c=NCORES, k=KT, p=P
                        )
                        for kb in range(KT):
                            nc.sync.dma_start(a1T[kb][:], src[kb])
                        for ch in range(4):
                            psq = ps2qk.tile([P, NCORES * P], F32, tag="psq")
                            for kb in range(KT):
                                for g in range(2):
                                    nc.tensor.matmul(
                                        psq[:, g * 512:(g + 1) * 512],
                                        wqkv_sb[kb][:, ch * P:(ch + 1) * P],
                                        a1T[kb][:, g * 512:(g + 1) * 512],
                                        start=(kb == 0), stop=(kb == KT - 1),
                                    )
                            nc.vector.tensor_copy(
                                qkT[ch][:, h * NCORES * P:
                                        (h + 1) * NCORES * P], psq[:]
                            )
                        if h == 0:
                            # scale prep off the sync queue (scalar DMAs)
                            nc.scalar.dma_start(
                                as1g[:],
                                ag1s_out.rearrange("(t p) o -> p (t o)", p=P)
                            )
                            as1row = p2m.tile([1, R], F32, tag="as1row")
                            nc.scalar.dma_start(
                                as1row[:], ag1s_out.rearrange("r o -> o r")
                            )
                            nc.vector.tensor_scalar(
                                as1row[:], as1row[:], cb[0][0:1, 0:1], None,
                                op0=ALU.mult
                            )
                            nc.gpsimd.partition_broadcast(cqb[:],
                                                          as1row[0:1, :])
                            nc.vector.tensor_scalar(
                                scv[:], as1g[:], cb[1][:, 0:1], None,
                                op0=ALU.mult
                            )
                        for c in range(NCORES):
                            t = 2 * c + h
                            psv = ps2v.tile([P, 2 * P], F32, tag="psv")
                            for kb in range(KT):
                                nc.tensor.matmul(
                                    psv[:], a1T[kb][:, c * P:(c + 1) * P],
                                    wqkv_sb[kb][:, 512:768],
                                    start=(kb == 0), stop=(kb == KT - 1),
                                )
                            nc.vector.tensor_scalar(
                                vplus[t][:, 0:128], psv[:, 0:128],
                                scv[:, t:t + 1], None, op0=ALU.mult,
                            )
                            nc.vector.tensor_scalar(
                                vplus[t][:, 129:257], psv[:, 128:256],
                                scv[:, t:t + 1], None, op0=ALU.mult,
                            )
                            nc.vector.memset(vplus[t][:, 128:129], 1.0)
                            nc.vector.memset(vplus[t][:, 257:258], 1.0)

            # ---- Phase 3: attention (quads) + o-proj + phase 4 ----
            with tc.tile_pool(name="p4w", bufs=1) as p4w, \
                 tc.tile_pool(name="p3x", bufs=1) as p3x:
                x_o = [p3x.tile([P, D], BF16, tag=f"xo{h}", name=f"xo{h}")
                       for h in range(LT)]
                aq_o = [p3x.tile([P, D], BF16, tag=f"aqo{h}", name=f"aqo{h}")
                        for h in range(LT)]
                olhsT = [p3x.tile([P, D], BF16, tag=f"olhsT{h}",
                                  name=f"olhsT{h}") for h in range(LT)]
                nw2 = p3x.tile([P, D], F32, tag="nw2")

                def attn_quad(q, s3, ps3s, ps3a):
                    b = [0, 1, 0, 1][q]
                    par = q // 2
                    imax = 6 + par
                    tiles = [b * 8 + 2 * s + par for s in range(4)]
                    qc0 = q * 4 * P
                    PT = {}
                    for hl in range(2):
                        S1 = [s3.tile([P, (2 * s + 2) * P], F32,
                                      tag=f"S1_{s}", name=f"S1_{s}", bufs=1)
                              for s in range(4)]
                        for j in range(imax + 1):
                            jt = b * 8 + j
                            psS = ps3s.tile([P, 4 * P], F32, tag="psS")
                            nc.tensor.matmul(
                                psS[:],
                                qkT[2 + hl][:, SLOT(jt) * P:(SLOT(jt) + 1) * P],
                                qkT[hl][:, qc0:qc0 + 4 * P],
                                start=True, stop=True,
                            )
                            for s in range(4):
                                i_s = 2 * s + par
                                if j > i_s:
                                    continue
                                nc.vector.scalar_tensor_tensor(
                                    S1[s][:, j * P:(j + 1) * P],
                                    psS[:, s * P:(s + 1) * P],
                                    as1g[:, jt:jt + 1],
                                    cqb[:, tiles[s] * P:(tiles[s] + 1) * P],
                                    op0=ALU.mult, op1=ALU.mult,
                                )
                                if j == i_s:
                                    nc.vector.tensor_tensor(
                                        S1[s][:, j * P:(j + 1) * P],
                                        S1[s][:, j * P:(j + 1) * P],
                                        maskT[:], op=ALU.add
                                    )
                                    L = (i_s + 1) * P
                                    pt = s3.tile([P, (2 * s + 2) * P], BF16,
                                                 tag=f"PT{hl}_{s}",
                                                 name=f"PT{hl}_{s}", bufs=1)
                                    nc.scalar.activation(
                                        pt[:, 0:L], S1[s][:, 0:L], AF.Exp
                                    )
                                    PT[(s, hl)] = pt
                    for s in range(4):
                        t = tiles[s]
                        i_s = 2 * s + par
                        anat = s3.tile([P, 2 * P], F32, tag="anat", bufs=3)
                        for hl in range(2):
                            att = ps3a.tile([P, 129], F32, tag="att")
                            for j in range(i_s + 1):
                                nc.tensor.matmul(
                                    att[:],
                                    PT[(s, hl)][:, j * P:(j + 1) * P],
                                    vplus[b * 8 + j][:, hl * 129:
                                                     (hl + 1) * 129],
                                    start=(j == 0), stop=(j == i_s),
                                )
                            erec = s3.tile([P, 1], F32, tag="erec", bufs=3)
                            nc.vector.reciprocal(erec[:], att[:, 128:129])
                            nc.vector.tensor_scalar(
                                anat[:, hl * P:(hl + 1) * P],
                                att[:, 0:128], erec[:, 0:1], None,
                                op0=ALU.mult,
                            )
                        ao = s3.tile([P, 2 * P], BF16, tag="ao", bufs=3)
                        nc.vector.tensor_copy(ao[:], anat[:])
                        c = t // 2
                        nc.sync.dma_start(
                            a2a_in[par][c * P:(c + 1) * P, :], ao[:]
                        )

                def oproj_quant_h(h, sp):
                    """Load A2A result, per-row amax, quantize (no PE)."""
                    nc.scalar.dma_start(
                        x_o[h][:],
                        a2a_out[h].rearrange("(s r) f -> r s f", s=NCORES)
                    )
                    am = sp.tile([P, 1], F32, tag="am_o", bufs=2)
                    nc.vector.tensor_reduce(
                        am[:], x_o[h][:], op=ALU.max, axis=AX.X,
                        apply_absolute_value=True,
                    )
                    nc.vector.tensor_scalar(
                        asol[:, h:h + 1], am[:], 1e-8, None, op0=ALU.add
                    )
                    inva = sp.tile([P, 1], F32, tag="inva_o", bufs=2)
                    nc.vector.reciprocal(inva[:], asol[:, h:h + 1])
                    nc.vector.tensor_scalar(
                        qs_o[:, h:h + 1], inva[:], 127.0, None, op0=ALU.mult
                    )
                    nc.vector.tensor_scalar(
                        sc_ol[:, h:h + 1], asol[:, h:h + 1], cb[2][:, 0:1],
                        None, op0=ALU.mult
                    )
                    _quant(nc, sp, x_o[h][:], qs_o[:, h:h + 1], aq_o[h][:],
                           tag="qotmp")

                def oproj_pe_h(h, wo_strips, sp, ps4t, ps5o, sq_pool):
                    for kb in range(KT):
                        pst = ps4t.tile([P, P], BF16, tag="pst4")
                        nc.tensor.transpose(
                            pst[:], aq_o[h][:, kb * P:(kb + 1) * P], ident[:]
                        )
                        nc.vector.tensor_copy(
                            olhsT[h][:, kb * P:(kb + 1) * P], pst[:]
                        )
                    po = ps5o.tile([P, D], F32, tag="po")
                    for kb in range(KT):
                        for n in range(4):
                            nc.tensor.matmul(
                                po[:, n * 512:(n + 1) * 512],
                                olhsT[h][:, kb * P:(kb + 1) * P],
                                wo_strips[kb][:, n * 512:(n + 1) * 512],
                                start=(kb == 0), stop=(kb == KT - 1),
                            )
                    xr = sp.tile([P, D], F32, tag="xr", bufs=1)
                    nc.sync.dma_start(xr[:], x_rows[h * P:(h + 1) * P, :])
                    nc.vector.scalar_tensor_tensor(
                        x1t[h][:], po[:], sc_ol[:, h:h + 1], xr[:],
                        op0=ALU.mult, op1=ALU.add,
                    )
                    nc.sync.dma_start(x1_d[h * P:(h + 1) * P, :], x1t[h][:])
                    # rmsnorm2 + quant + transpose + stage AG2 chunk h
                    as_l2 = sp.tile([P, 1], F32, tag="as_l2", bufs=2)
                    aq2 = sp.tile([P, D], BF16, tag="aq2", bufs=2)
                    _rms_quant_rows(nc, sp, sq_pool, x1t[h], nw2,
                                    as_l2[:, 0:1], aq2[:])
                    nc.sync.dma_start(ag2s_in[h * P:(h + 1) * P, :], as_l2[:])
                    if h == 1:
                        # scales AG between the two act chunks
                        nc.gpsimd.collective_compute(
                            "AllGather", ALU.bypass, replica_groups=rg,
                            ins=[ag2s_in.opt()], outs=[ag2s_out.opt()],
                        )
                    for kb in range(KT):
                        pst = ps4t.tile([P, P], BF16, tag="pst4")
                        nc.tensor.transpose(
                            pst[:], aq2[:, kb * P:(kb + 1) * P], ident[:]
                        )
                        aqs2 = sp.tile([P, P], BF16, tag="aqs2", bufs=3)
                        nc.vector.tensor_copy(aqs2[:], pst[:])
                        nc.sync.dma_start(
                            ag2c_in[h][kb * P:(kb + 1) * P, :], aqs2[:]
                        )
                    nc.gpsimd.collective_compute(
                        "AllGather", ALU.bypass, replica_groups=rg,
                        ins=[ag2c_in[h].opt()], outs=[ag2c_out[h].opt()],
                    )

                with tc.tile_pool(name="p5sc", bufs=2) as s5:
                    nwr2 = s5.tile([1, D], F32, tag="nwr2", bufs=1)
                    nc.scalar.dma_start(nwr2[:], norm2_w)
                    nc.gpsimd.partition_broadcast(nw2[:], nwr2[0:1, :])
                    wo0 = []
                    with tc.tile_pool(name="p3sc", bufs=4) as s3, \
                         tc.tile_pool(name="ps3s", bufs=4,
                                      space="PSUM") as ps3s, \
                         tc.tile_pool(name="ps3a", bufs=4,
                                      space="PSUM") as ps3a:
                        attn_quad(0, s3, ps3s, ps3a)
                        attn_quad(1, s3, ps3s, ps3a)
                        nc.gpsimd.collective_compute(
                            "AllToAll", ALU.bypass, replica_groups=rg,
                            ins=[a2a_in[0].opt()], outs=[a2a_out[0].opt()],
                        )
                        attn_quad(2, s3, ps3s, ps3a)
                        # prefetch o-proj weights for h=0 during attention
                        for kb in range(KT):
                            wt = p4w.tile([P, D], BF16, tag="wo_t", bufs=8)
                            nc.sync.dma_start(
                                wt[:], woT[kb * P:(kb + 1) * P, :]
                            )
                            wo0.append(wt)
                        oproj_quant_h(0, s5)
                        attn_quad(3, s3, ps3s, ps3a)
                        nc.gpsimd.collective_compute(
                            "AllToAll", ALU.bypass, replica_groups=rg,
                            ins=[a2a_in[1].opt()], outs=[a2a_out[1].opt()],
                        )
                        oproj_quant_h(1, s5)
                    with tc.tile_pool(name="ps4t", bufs=2,
                                      space="PSUM") as ps4t, \
                         tc.tile_pool(name="ps5o", bufs=1,
                                      space="PSUM") as ps5o:
                        sqp = s5.tile([P, D], F32, tag="sqp")
                        sq_pool = _FixedTilePool(sqp)
                        oproj_pe_h(0, wo0, s5, ps4t, ps5o, sq_pool)
                        wo1 = []
                        for kb in range(KT):
                            wt = p4w.tile([P, D], BF16, tag="wo_t", bufs=8)
                            nc.sync.dma_start(
                                wt[:], woT[kb * P:(kb + 1) * P, :]
                            )
                            wo1.append(wt)
                        oproj_pe_h(1, wo1, s5, ps4t, ps5o, sq_pool)
        # mega-pool 1 frees here

        # =========================================================
        # Mega-pool 2: MLP (phases 5,6)
        # =========================================================
        NG = 4
        NOC = 4
        OCW = D // NOC  # 512
        proc = [2 * c + h for h in range(LT) for c in range(NCORES)]
        with tc.tile_pool(name="mp2", bufs=1) as m2, \
             tc.tile_pool(name="mp2sc", bufs=2) as s6:
            wgu_sb = [m2.tile([P, 2 * OM], BF16, tag=f"wgu{k}", name=f"wgu{k}")
                      for k in range(KT)]
            for k in range(KT):
                nc.sync.dma_start(wgu_sb[k][:], wguT[k * P:(k + 1) * P, :])

            nc.scalar.dma_start(
                as2g[:], ag2s_out.rearrange("(t p) o -> p (t o)", p=P)
            )
            nc.vector.tensor_scalar(
                sc_g[:], as2g[:], cb[3][:, 0:1], None, op0=ALU.mult
            )
            nc.vector.tensor_scalar(
                sc_u[:], as2g[:], cb[4][:, 0:1], None, op0=ALU.mult
            )
            # one-hot row-select (host input): picks this core's own tile
            # column out of the AR'd scale tiles -> local down-proj scale,
            # replacing v2's ReduceScatter(max)
            rsel = m2.tile([P, NCORES], F32, tag="rsel")
            rselr = s6.tile([1, NCORES], F32, tag="rselr", bufs=1)
            nc.scalar.dma_start(rselr[:], rowsel_d)
            nc.gpsimd.partition_broadcast(rsel[:], rselr[0:1, :])

            asm_in = [dp.tile([4 * P, 1], F32, tag=f"asmi{g}", name=f"asmi{g}")
                      for g in range(NG)]
            asm_go = [dp.tile([4 * P, 1], F32, tag=f"asmo{g}", name=f"asmo{g}",
                              addr_space="Shared") for g in range(NG)]
            mT = [m2.tile([P, R], BF16, tag=f"mT{kb}", name=f"mT{kb}")
                  for kb in range(MT)]
            m_tiles = [m2.tile([P, OM], BF16, tag=f"m{t}", name=f"m{t}")
                       for t in range(RT)]
            a2t_src = [ag2c_out[h].rearrange(
                "(c k p) j -> c p k j", c=NCORES, k=KT, p=P)
                for h in range(LT)]
            x1r = [m2.tile([P, D], F32, tag=f"x1r{lt}", name=f"x1r{lt}")
                   for lt in range(LT)]
            for lt in range(LT):
                nc.sync.dma_start(x1r[lt][:], x1_d[lt * P:(lt + 1) * P, :])
            rs_in = [dp.tile([NCORES * P, OCW], BF16, tag=f"rsdi{i}",
                             name=f"rsdi{i}") for i in range(NOC * LT)]
            rs_out = [dp.tile([P, OCW], BF16, tag=f"rsdo{i}",
                              name=f"rsdo{i}") for i in range(NOC * LT)]

            def gate_up_compute(g, ps6):
                for tl in range(4):
                    pi = g * 4 + tl
                    t = proc[pi]
                    a2t = s6.tile([P, D], BF16, tag="a2t", bufs=2)
                    nc.sync.dma_start(a2t[:], a2t_src[t % 2][t // 2])
                    psg = []
                    for n in range(4):
                        pg = ps6.tile([P, 512], F32, tag="psg", bufs=5)
                        for kb in range(KT):
                            nc.tensor.matmul(
                                pg[:], a2t[:, kb * P:(kb + 1) * P],
                                wgu_sb[kb][:, n * 512:(n + 1) * 512],
                                start=(kb == 0), stop=(kb == KT - 1),
                            )
                        psg.append(pg)
                    for half in range(2):
                        # silu(g_deq) = g_deq * sigmoid(g_deq)
                        sig = s6.tile([P, 512], F32, tag="sig")
                        nc.scalar.activation(
                            sig[:], psg[half][:], AF.Sigmoid,
                            scale=sc_g[:, t:t + 1]
                        )
                        sgl = s6.tile([P, 512], F32, tag="sgl")
                        nc.vector.scalar_tensor_tensor(
                            sgl[:], psg[half][:], sc_g[:, t:t + 1],
                            sig[:], op0=ALU.mult, op1=ALU.mult,
                        )
                        nc.vector.scalar_tensor_tensor(
                            m_tiles[t][:, half * 512:(half + 1) * 512],
                            psg[2 + half][:], sc_u[:, t:t + 1], sgl[:],
                            op0=ALU.mult, op1=ALU.mult,
                        )
                    nc.vector.tensor_reduce(
                        asm[:, pi:pi + 1], m_tiles[t][:], op=ALU.max,
                        axis=AX.X, apply_absolute_value=True,
                    )
                gs = slice(g * 4, (g + 1) * 4)
                nc.sync.dma_start(
                    asm_in[g].rearrange("(t p) o -> p (t o)", p=P),
                    asm[:, gs],
                )
                nc.gpsimd.collective_compute(
                    "AllReduce", ALU.max, replica_groups=rg,
                    ins=[asm_in[g].opt()], outs=[asm_go[g].opt()],
                )

            def post_ar(g, ps6):
                # emitted >=1 group after the AR fire, so none of these
                # queue instructions ever waits on an in-flight collective
                gs = slice(g * 4, (g + 1) * 4)
                nc.scalar.dma_start(
                    asmg[:, gs],
                    asm_go[g].rearrange("(t p) o -> p (t o)", p=P),
                )
                nc.vector.tensor_scalar(
                    asmg[:, gs], asmg[:, gs], 1e-8, None, op0=ALU.add
                )
                nc.vector.reciprocal(qsm[:, gs], asmg[:, gs])
                nc.vector.tensor_scalar(
                    qsm[:, gs], qsm[:, gs], 127.0, None, op0=ALU.mult
                )
                for tl in range(4):
                    pi = g * 4 + tl
                    t = proc[pi]
                    m_q = s6.tile([P, OM], BF16, tag="m_q", bufs=2)
                    _quant(nc, s6, m_tiles[t][:], qsm[:, pi:pi + 1],
                           m_q[:], tag="qtmp")
                    if g < NG - 1:
                        for kb in range(MT):
                            eng = nc.sync if kb % 2 == 0 else nc.scalar
                            eng.dma_start(
                                mT[kb][:, t * P:(t + 1) * P],
                                m_q[:, kb * P:(kb + 1) * P], transpose=True,
                            )
                    else:
                        # last group: PE transposes, so down-proj odds do
                        # not wait ~40us for DMA transposes
                        for kb in range(MT):
                            pst = ps6.tile([P, P], BF16, tag="pst6", bufs=1)
                            nc.tensor.transpose(
                                pst[:], m_q[:, kb * P:(kb + 1) * P], ident[:]
                            )
                            nc.vector.tensor_copy(
                                mT[kb][:, t * P:(t + 1) * P], pst[:]
                            )

            def sc_dl_half(lt):
                # sc_dl[:, lt] = asmg[pi of own tile 2c+lt] * wsd/127 via
                # one-hot mask (columns lt*8..lt*8+8 are all written by now)
                tmpm = s6.tile([P, NCORES], F32, tag="sctmp", bufs=1)
                nc.vector.tensor_tensor(
                    tmpm[:], asmg[:, lt * 8:(lt + 1) * 8], rsel[:],
                    op=ALU.mult
                )
                scr = s6.tile([P, 1], F32, tag="scr", bufs=1)
                nc.vector.tensor_reduce(
                    scr[:], tmpm[:], op=ALU.max, axis=AX.X
                )
                nc.vector.tensor_scalar(
                    sc_dl[:, lt:lt + 1], scr[:], cb[5][:, 0:1], None,
                    op0=ALU.mult
                )

            def down_pass(par, p7w, ps6):
                for oc in range(NOC):
                    wd_oc = p7w.tile([P, MT * OCW], BF16, tag="wd_oc")
                    for kb in range(MT):
                        nc.sync.dma_start(
                            wd_oc[:, kb * OCW:(kb + 1) * OCW],
                            wdT[kb * P:(kb + 1) * P,
                                oc * OCW:(oc + 1) * OCW],
                        )
                    ci = oc * 2 + par
                    for c in range(NCORES):
                        t = 2 * c + par
                        pso = ps6.tile([P, OCW], F32, tag="dps", bufs=2)
                        for kb in range(MT):
                            nc.tensor.matmul(
                                pso[:], mT[kb][:, t * P:(t + 1) * P],
                                wd_oc[:, kb * OCW:(kb + 1) * OCW],
                                start=(kb == 0), stop=(kb == MT - 1),
                            )
                        # bf16 RS halves wire traffic; partials are ints
                        # < 2^18 so bf16 adds ~2^-9 relative rounding
                        dsb = s6.tile([P, OCW], BF16, tag="dsb", bufs=3)
                        nc.vector.tensor_copy(dsb[:], pso[:])
                        nc.sync.dma_start(
                            rs_in[ci][c * P:(c + 1) * P, :], dsb[:]
                        )
                    nc.gpsimd.collective_compute(
                        "ReduceScatter", ALU.add, replica_groups=rg,
                        ins=[rs_in[ci].opt()], outs=[rs_out[ci].opt()],
                    )

            def down_drain(par):
                for oc in range(NOC):
                    ci = oc * 2 + par
                    ysb = s6.tile([P, OCW], BF16, tag="ysb")
                    nc.scalar.dma_start(ysb[:], rs_out[ci][:])
                    ot = s6.tile([P, OCW], F32, tag="ot")
                    nc.vector.scalar_tensor_tensor(
                        ot[:], ysb[:], sc_dl[:, par:par + 1],
                        x1r[par][:, oc * OCW:(oc + 1) * OCW],
                        op0=ALU.mult, op1=ALU.add,
                    )
                    nc.sync.dma_start(
                        out_d[par * P:(par + 1) * P,
                              oc * OCW:(oc + 1) * OCW],
                        ot[:],
                    )

            with tc.tile_pool(name="ps6", bufs=1, space="PSUM") as ps6, \
                 tc.tile_pool(name="p7w", bufs=2) as p7w:
                gate_up_compute(0, ps6)
                gate_up_compute(1, ps6)
                post_ar(0, ps6)
                gate_up_compute(2, ps6)
                post_ar(1, ps6)
                sc_dl_half(0)
                # down-proj even row tiles: their mT strips (groups 0-1)
                # are transposed by now; the 4 RS-evens overlap group 3
                down_pass(0, p7w, ps6)
                post_ar(2, ps6)
                gate_up_compute(3, ps6)
                post_ar(3, ps6)
                sc_dl_half(1)
                down_drain(0)
                down_pass(1, p7w, ps6)
                down_drain(1)

    nc.compile()
    return nc


def _ternary_quant(w):
    """BitNet weight quant on host: ternary bf16 (exact) + f32 scale."""
    ws = float(np.abs(np.asarray(w, np.float64)).mean()) + 1e-8
    q = np.clip(np.round(np.asarray(w, np.float64) / ws), -1.0, 1.0)
    return q.astype(ml_dtypes.bfloat16), np.float32(ws)


def _prep_in_maps(inputs):
    x = np.asarray(inputs["x"], np.float32).reshape(R, D)
    wq, wsq = _ternary_quant(inputs["wq"])
    wk, wsk = _ternary_quant(inputs["wk"])
    wv, wsv = _ternary_quant(inputs["wv"])
    wo, wso = _ternary_quant(inputs["wo"])
    wg, wsg = _ternary_quant(inputs["wg"])
    wu, wsu = _ternary_quant(inputs["wu"])
    wd, wsd = _ternary_quant(inputs["wd"])
    n1 = np.asarray(inputs["norm1_w"], np.float32).reshape(1, D)
    n2 = np.asarray(inputs["norm2_w"], np.float32).reshape(1, D)

    ident = np.eye(P, dtype=ml_dtypes.bfloat16)
    iv, jv = np.mgrid[0:P, 0:P]
    # transposed causal block mask: key row u > query col v is masked
    causal_t = np.where(iv <= jv, 0.0, -1e30).astype(np.float32)
    wconsts = np.array([[
        wsq * wsk * INV_SQRT_HD / (127.0 * 127.0),
        wsv / 127.0, wso / 127.0, wsg / 127.0, wsu / 127.0, wsd / 127.0,
        0.0, 0.0,
    ]], np.float32)
    woT_full = np.ascontiguousarray(wo.T)

    in_maps = []
    for c in range(NCORES):
        qs = slice(c * OQ, (c + 1) * OQ)
        ms = slice(c * OM, (c + 1) * OM)
        # per-head-major qkv columns: q-h0, q-h1, k-h0, k-h1, v-h0, v-h1
        rowsel = np.zeros((1, NCORES), np.float32)
        rowsel[0, c] = 1.0
        in_maps.append({
            "x_rows": np.ascontiguousarray(x[c * RL:(c + 1) * RL]),
            "rowsel": rowsel,
            "wqkvT": np.ascontiguousarray(
                np.concatenate([wq[qs], wk[qs], wv[qs]], 0).T
            ),
            "woT": woT_full,
            "wguT": np.ascontiguousarray(
                np.concatenate([wg[ms], wu[ms]], 0).T
            ),
            "wdT": np.ascontiguousarray(wd[:, ms].T),
            "norm1_w": n1,
            "norm2_w": n2,
            "ident_b": ident,
            "causal_t": causal_t,
            "wconsts": wconsts,
        })
    return in_maps


def kernel(**inputs) -> np.ndarray:
    global _CACHED_NC
    if _CACHED_NC is None:
        _CACHED_NC = build_program()
    nc = _CACHED_NC
    in_maps = _prep_in_maps(inputs)
    res = run_bass_kernel_spmd(nc, in_maps, core_ids=list(range(NCORES)))
    out = np.concatenate([res.results[c]["out"] for c in range(NCORES)], 0)
    return out.reshape(B, S, D).astype(np.float32)


# revision 34
# speedup vs baseline: 1.1536x; 1.1536x over previous
"""BitNet transformer block on 8 Trainium2 NeuronCores (Megatron tensor-parallel).

Self-contained: builds one SPMD Bass/Tile program, shards inputs on host,
runs via run_bass_kernel_spmd, gathers the output.

v3 design (vs v2): eliminate tensor-engine idle (v2 trace: PE union-busy
560us of 1045us span; ~440us of idle gaps around collectives).
  - AG1/AG2 of activations split into two 512KB column chunks (one per local
    row tile); QKV / gate-up process row-chunks as they arrive.
  - Attention scores batched: 4 query tiles per matmul (N=512) against each
    key tile. Query tiles live in qkT in chunk-slot order (evens then odds)
    so quads are contiguous.
  - Attention outputs are AllToAll'd RAW (bf16, natural [rows, feats]
    layout) in two chunks fired mid-attention; the int8-grid quantization
    for o-proj happens AFTER the A2A where each core owns all features of
    its rows, so the per-row a_scale is local. This removes v2's four
    2KB AllReduce(max) ops (12-25us each) and the 8KB ReduceScatter.
  - o-proj / x1 / rmsnorm2 / AG2 pipelined per row tile.
  - MLP: m tiles kept in bf16 so all 16 stay resident (no buffer
    starvation around the scale AllReduces); gate/up uses [P,512] PSUM
    chunks; down-proj ReduceScatter split per (oc, even/odd row tiles)
    into 8 x 1MB chunks so the exposed tail is one 1MB RS.

Numerics: quantized activations (ints in [-127,127]) and ternary weights are
exact in bf16; matmuls accumulate in fp32 PSUM, so every BitNet matmul is
exact integer arithmetic. Attention outputs and m cross the wire / live in
bf16 before their quantization: this adds <=0.25 int-step of extra rounding
on top of the inherent 0.5-step quant noise. Rounding uses the fp32
magic-constant trick (+1.5*2^23) matching jnp.round ties-to-even.
"""

import os

import numpy as np
import ml_dtypes

import concourse.bacc as bacc
import concourse.mybir as mybir
import concourse.tile as tile
from concourse.bass_utils import run_bass_kernel_spmd

F32 = mybir.dt.float32
BF16 = mybir.dt.bfloat16
AF = mybir.ActivationFunctionType
ALU = mybir.AluOpType
AX = mybir.AxisListType

NCORES = 8
B, S, D, H, MLP = 2, 1024, 2048, 16, 8192
HD = 128
R = B * S                 # 2048 rows total
RL = R // NCORES          # 256 rows per core (row shard)
OQ = D // NCORES          # 256 qkv out cols per core (2 heads)
OM = MLP // NCORES        # 1024 mlp cols per core
P = 128
KT = D // P               # 16 feature chunks
RT = R // P               # 16 row tiles
LT = RL // P              # 2 local row tiles
ST = S // P               # 8 seq tiles per batch
MT = OM // P              # 8 mlp k-chunks per core
MAGIC = 12582912.0        # 1.5 * 2**23: fp32 round-to-nearest-even magic
INV_SQRT_HD = 1.0 / float(np.sqrt(HD))

_CACHED_NC = None


def SLOT(t):
    """qkT column slot for global row tile t (evens first, then odds)."""
    return (t % 2) * 8 + t // 2


def _quant(nc, sp, src_ap, qscale_ap, out_bf_ap, tag="qtmp"):
    """out_bf = round(src * qscale) as bf16.

    fp32 +MAGIC rounds to integer (RNE); ACT subtracts MAGIC and casts to
    bf16 (small ints are exact in bf16).
    """
    F = src_ap.shape[1]
    CH = min(F, 1024)  # bound the fp32 scratch to 4KB/partition
    for c0 in range(0, F, CH):
        tmp = sp.tile([src_ap.shape[0], CH], F32, tag=tag, name=tag)
        nc.vector.tensor_scalar(
            tmp[:], src_ap[:, c0:c0 + CH], qscale_ap, MAGIC,
            op0=ALU.mult, op1=ALU.add,
        )
        nc.scalar.activation(
            out_bf_ap[:, c0:c0 + CH], tmp[:], AF.Copy, bias=-MAGIC, scale=1.0
        )


class _FixedTilePool:
    """Adapter handing out a pre-allocated tile (for _rms_quant_rows sqd)."""

    def __init__(self, t):
        self._t = t

    def tile(self, shape, dtype, tag=""):
        return self._t


def _rms_quant_rows(nc, sp, ps_dummy, src_tile, nw_tile, as_out_ap, aq_out_ap):
    """rmsnorm + abs-max + int8-grid quantize for one [128, D] row tile.

    Writes a_scale (max|h|+1e-8, h = src/rms*nw) to as_out_ap [128,1] and
    the quantized bf16 ints to aq_out_ap. Mutates src_tile in place
    (src *= nw).
    """
    sqd = ps_dummy.tile([P, D], F32, tag="sqd")
    ssq = sp.tile([P, 1], F32, tag="ssq")
    nc.scalar.activation(sqd[:], src_tile[:], AF.Square, accum_out=ssq[:])
    rms = sp.tile([P, 1], F32, tag="rms")
    nc.vector.tensor_scalar(
        rms[:], ssq[:], 1.0 / D, 1e-6, op0=ALU.mult, op1=ALU.add
    )
    nc.scalar.activation(rms[:], rms[:], AF.Sqrt)
    rinv = sp.tile([P, 1], F32, tag="rinv")
    nc.vector.reciprocal(rinv[:], rms[:])
    nc.vector.tensor_tensor(src_tile[:], src_tile[:], nw_tile[:], op=ALU.mult)
    amax = sp.tile([P, 1], F32, tag="amax")
    nc.vector.tensor_reduce(
        amax[:], src_tile[:], op=ALU.max, axis=AX.X, apply_absolute_value=True
    )
    nc.vector.tensor_scalar(
        as_out_ap, amax[:], rinv[:], 1e-8, op0=ALU.mult, op1=ALU.add
    )
    inva = sp.tile([P, 1], F32, tag="inva")
    nc.vector.reciprocal(inva[:], as_out_ap)
    qs = sp.tile([P, 1], F32, tag="qs")
    nc.vector.tensor_scalar(
        qs[:], inva[:], rinv[:], 127.0, op0=ALU.mult, op1=ALU.mult
    )
    _quant(nc, sp, src_tile[:], qs[:, 0:1], aq_out_ap)


def build_program():
    nc = bacc.Bacc(
        "TRN2",
        target_bir_lowering=False,
        debug=False,
        enable_asserts=True,
        num_devices=NCORES,
    )
    rg = [list(range(NCORES))]

    # ---------------- I/O (identical layouts to v2) ----------------
    x_rows = nc.dram_tensor("x_rows", [RL, D], F32, kind="ExternalInput").ap()
    wqkvT = nc.dram_tensor("wqkvT", [D, 3 * OQ], BF16, kind="ExternalInput").ap()
    woT = nc.dram_tensor("woT", [D, D], BF16, kind="ExternalInput").ap()
    wguT = nc.dram_tensor("wguT", [D, 2 * OM], BF16, kind="ExternalInput").ap()
    wdT = nc.dram_tensor("wdT", [OM, D], BF16, kind="ExternalInput").ap()
    norm1_w = nc.dram_tensor("norm1_w", [1, D], F32, kind="ExternalInput").ap()
    norm2_w = nc.dram_tensor("norm2_w", [1, D], F32, kind="ExternalInput").ap()
    ident_b = nc.dram_tensor("ident_b", [P, P], BF16, kind="ExternalInput").ap()
    causal_t = nc.dram_tensor("causal_t", [P, P], F32, kind="ExternalInput").ap()
    wconsts = nc.dram_tensor("wconsts", [1, 8], F32, kind="ExternalInput").ap()
    rowsel_d = nc.dram_tensor("rowsel", [1, NCORES], F32,
                              kind="ExternalInput").ap()
    out_d = nc.dram_tensor("out", [RL, D], F32, kind="ExternalOutput").ap()

    with tile.TileContext(nc) as tc, \
         tc.tile_pool(name="persist", bufs=1) as pp, \
         tc.tile_pool(name="dram", bufs=1, space="DRAM") as dp:

        # ---------------- constants ----------------
        ident = pp.tile([P, P], BF16, tag="ident")
        nc.sync.dma_start(ident[:], ident_b)
        maskT = pp.tile([P, P], F32, tag="maskT")
        nc.sync.dma_start(maskT[:], causal_t)
        wcrow = pp.tile([1, 8], F32, tag="wcrow")
        nc.sync.dma_start(wcrow[:], wconsts)
        # 0: wsq*wsk/(127^2 sqrt(HD)), 1: wsv/127, 2: wso/127,
        # 3: wsg/127, 4: wsu/127, 5: wsd/127
        cb = {}
        for slot in range(6):
            cb[slot] = pp.tile([P, 1], F32, tag=f"cb{slot}", name=f"cb{slot}")
            nc.gpsimd.partition_broadcast(
                cb[slot][:], wcrow[0:1, slot:slot + 1]
            )

        # persistent scale tiles
        as1g = pp.tile([P, RT], F32, tag="as1g")
        scv = pp.tile([P, RT], F32, tag="scv")
        asol = pp.tile([P, LT], F32, tag="asol")
        sc_ol = pp.tile([P, LT], F32, tag="sc_ol")
        qs_o = pp.tile([P, LT], F32, tag="qs_o")
        as2g = pp.tile([P, RT], F32, tag="as2g")
        sc_g = pp.tile([P, RT], F32, tag="sc_g")
        sc_u = pp.tile([P, RT], F32, tag="sc_u")
        asm = pp.tile([P, RT], F32, tag="asm")      # proc-order columns
        asmg = pp.tile([P, RT], F32, tag="asmg")
        qsm = pp.tile([P, RT], F32, tag="qsm")
        asml = pp.tile([P, LT], F32, tag="asml")
        sc_dl = pp.tile([P, LT], F32, tag="sc_dl")

        # collective DRAM buffers
        ag1c_in = [dp.tile([D, P], BF16, tag=f"ag1ci{h}", name=f"ag1ci{h}")
                   for h in range(LT)]
        ag1c_out = [dp.tile([NCORES * D, P], BF16, tag=f"ag1co{h}",
                            name=f"ag1co{h}", addr_space="Shared")
                    for h in range(LT)]
        ag1s_in = dp.tile([RL, 1], F32, tag="ag1s_in")
        ag1s_out = dp.tile([R, 1], F32, tag="ag1s_out", addr_space="Shared")
        a2a_in = [dp.tile([NCORES * P, 2 * P], BF16, tag=f"a2ai{h}",
                          name=f"a2ai{h}") for h in range(LT)]
        a2a_out = [dp.tile([NCORES * P, 2 * P], BF16, tag=f"a2ao{h}",
                           name=f"a2ao{h}") for h in range(LT)]
        x1_d = dp.tile([RL, D], F32, tag="x1_d")
        ag2c_in = [dp.tile([D, P], BF16, tag=f"ag2ci{h}", name=f"ag2ci{h}")
                   for h in range(LT)]
        ag2c_out = [dp.tile([NCORES * D, P], BF16, tag=f"ag2co{h}",
                            name=f"ag2co{h}", addr_space="Shared")
                    for h in range(LT)]
        ag2s_in = dp.tile([RL, 1], F32, tag="ag2s_in")
        ag2s_hout = [dp.tile([NCORES * P, 1], F32, tag=f"ag2sh{h}",
                             name=f"ag2sh{h}", addr_space="Shared")
                     for h in range(LT)]

        # =========================================================
        # Mega-pool 1: phases 1,2,3 + o-proj + phase 4
        # =========================================================
        with tc.tile_pool(name="mp1", bufs=1) as m1:
            nw1 = m1.tile([P, D], F32, tag="nw1")
            cqb = m1.tile([P, R], F32, tag="cqb")
            qkT = [m1.tile([P, R], BF16, tag=f"qkT{ch}", name=f"qkT{ch}")
                   for ch in range(4)]
            vplus = [m1.tile([P, 258], BF16, tag=f"vp{t}", name=f"vp{t}")
                     for t in range(RT)]
            x1t = [m1.tile([P, D], F32, tag=f"x1t{lt}", name=f"x1t{lt}")
                   for lt in range(LT)]

            # ---- Phase 1: local rmsnorm1 + quant + transpose + chunked AG
            with tc.tile_pool(name="p2w", bufs=1) as p2m:
                wqkv_sb = [p2m.tile([P, 3 * OQ], BF16, tag=f"wqkv{k}",
                                    name=f"wqkv{k}") for k in range(KT)]
                with tc.tile_pool(name="p1sc", bufs=2) as s1, \
                     tc.tile_pool(name="ps1", bufs=1, space="PSUM") as ps1, \
                     tc.tile_pool(name="ps1t", bufs=2, space="PSUM") as ps1t:
                    # pre-emit x loads (sync); weight prefetch on scalar queue
                    xt = [s1.tile([P, D], F32, tag=f"xt{lt}", name=f"xt{lt}",
                                  bufs=1) for lt in range(LT)]
                    for lt in range(LT):
                        nc.sync.dma_start(
                            xt[lt][:], x_rows[lt * P:(lt + 1) * P, :]
                        )
                    for k in range(KT):
                        nc.scalar.dma_start(
                            wqkv_sb[k][:], wqkvT[k * P:(k + 1) * P, :]
                        )
                    nwr = s1.tile([1, D], F32, tag="nwr", bufs=1)
                    nc.sync.dma_start(nwr[:], norm1_w)
                    nc.gpsimd.partition_broadcast(nw1[:], nwr[0:1, :])
                    for lt in range(LT):
                        as_l = s1.tile([P, 1], F32, tag="as_l")
                        aq = s1.tile([P, D], BF16, tag="aq", bufs=1)
                        _rms_quant_rows(nc, s1, ps1, xt[lt], nw1,
                                        as_l[:, 0:1], aq[:])
                        nc.sync.dma_start(
                            ag1s_in[lt * P:(lt + 1) * P, :], as_l[:]
                        )
                        for kb in range(KT):
                            pst = ps1t.tile([P, P], BF16, tag="pst")
                            nc.tensor.transpose(
                                pst[:], aq[:, kb * P:(kb + 1) * P], ident[:]
                            )
                            aqs = s1.tile([P, P], BF16, tag="aqs", bufs=3)
                            nc.vector.tensor_copy(aqs[:], pst[:])
                            nc.sync.dma_start(
                                ag1c_in[lt][kb * P:(kb + 1) * P, :], aqs[:]
                            )
                        if lt == 1:
                            # scales AG between the two act chunks (its input
                            # is complete once both as_l DMAs have landed)
                            nc.gpsimd.collective_compute(
                                "AllGather", ALU.bypass, replica_groups=rg,
                                ins=[ag1s_in.opt()], outs=[ag1s_out.opt()],
                            )
                        nc.gpsimd.collective_compute(
                            "AllGather", ALU.bypass, replica_groups=rg,
                            ins=[ag1c_in[lt].opt()], outs=[ag1c_out[lt].opt()],
                        )

                # ---- Phase 2: QKV per AG chunk ----
                # qkT[ch] columns in chunk-slot order: tile t at SLOT(t)*P
                with tc.tile_pool(name="ps2qk", bufs=2, space="PSUM") as ps2qk, \
                     tc.tile_pool(name="ps2v", bufs=2, space="PSUM") as ps2v:
                    for h in range(LT):
                        a1T = [p2m.tile([P, NCORES * P], BF16,
                                        tag=f"a1T{kb}", name=f"a1T{kb}",
                                        bufs=2) for kb in range(KT)]
                        src = ag1c_out[h].rearrange(
                            "(c k p) j -> k p c j", c=NCORES, k=KT, p=P
                        )
                        for kb in range(KT):
                            nc.sync.dma_start(a1T[kb][:], src[kb])
                        for ch in range(4):
                            psq = ps2qk.tile([P, NCORES * P], F32, tag="psq")
                            for kb in range(KT):
                                for g in range(2):
                                    nc.tensor.matmul(
                                        psq[:, g * 512:(g + 1) * 512],
                                        wqkv_sb[kb][:, ch * P:(ch + 1) * P],
                                        a1T[kb][:, g * 512:(g + 1) * 512],
                                        start=(kb == 0), stop=(kb == KT - 1),
                                    )
                            nc.vector.tensor_copy(
                                qkT[ch][:, h * NCORES * P:
                                        (h + 1) * NCORES * P], psq[:]
                            )
                        if h == 0:
                            # scale prep off the sync queue (scalar DMAs)
                            nc.scalar.dma_start(
                                as1g[:],
                                ag1s_out.rearrange("(t p) o -> p (t o)", p=P)
                            )
                            as1row = p2m.tile([1, R], F32, tag="as1row")
                            nc.scalar.dma_start(
                                as1row[:], ag1s_out.rearrange("r o -> o r")
                            )
                            nc.vector.tensor_scalar(
                                as1row[:], as1row[:], cb[0][0:1, 0:1], None,
                                op0=ALU.mult
                            )
                            nc.gpsimd.partition_broadcast(cqb[:],
                                                          as1row[0:1, :])
                            nc.vector.tensor_scalar(
                                scv[:], as1g[:], cb[1][:, 0:1], None,
                                op0=ALU.mult
                            )
                        for c in range(NCORES):
                            t = 2 * c + h
                            psv = ps2v.tile([P, 2 * P], F32, tag="psv")
                            for kb in range(KT):
                                nc.tensor.matmul(
                                    psv[:], a1T[kb][:, c * P:(c + 1) * P],
                                    wqkv_sb[kb][:, 512:768],
                                    start=(kb == 0), stop=(kb == KT - 1),
                                )
                            nc.vector.tensor_scalar(
                                vplus[t][:, 0:128], psv[:, 0:128],
                                scv[:, t:t + 1], None, op0=ALU.mult,
                            )
                            nc.vector.tensor_scalar(
                                vplus[t][:, 129:257], psv[:, 128:256],
                                scv[:, t:t + 1], None, op0=ALU.mult,
                            )
                            nc.vector.memset(vplus[t][:, 128:129], 1.0)
                            nc.vector.memset(vplus[t][:, 257:258], 1.0)

            # ---- Phase 3: attention (quads) + o-proj + phase 4 ----
            with tc.tile_pool(name="p4w", bufs=1) as p4w, \
                 tc.tile_pool(name="p3x", bufs=1) as p3x:
                x_o = [p3x.tile([P, D], BF16, tag=f"xo{h}", name=f"xo{h}")
                       for h in range(LT)]
                aq_o = [p3x.tile([P, D], BF16, tag=f"aqo{h}", name=f"aqo{h}")
                        for h in range(LT)]
                olhsT = [p3x.tile([P, D], BF16, tag=f"olhsT{h}",
                                  name=f"olhsT{h}") for h in range(LT)]
                nw2 = p3x.tile([P, D], F32, tag="nw2")

                def attn_quad(q, s3, ps3s, ps3a):
                    b = [0, 1, 0, 1][q]
                    par = q // 2
                    imax = 6 + par
                    tiles = [b * 8 + 2 * s + par for s in range(4)]
                    qc0 = q * 4 * P
                    PT = {}
                    for hl in range(2):
                        S1 = [s3.tile([P, (2 * s + 2) * P], F32,
                                      tag=f"S1_{s}", name=f"S1_{s}", bufs=2)
                              for s in range(4)]
                        for j in range(imax + 1):
                            jt = b * 8 + j
                            psS = ps3s.tile([P, 4 * P], F32, tag="psS")
                            nc.tensor.matmul(
                                psS[:],
                                qkT[2 + hl][:, SLOT(jt) * P:(SLOT(jt) + 1) * P],
                                qkT[hl][:, qc0:qc0 + 4 * P],
                                start=True, stop=True,
                            )
                            for s in range(4):
                                i_s = 2 * s + par
                                if j > i_s:
                                    continue
                                nc.vector.scalar_tensor_tensor(
                                    S1[s][:, j * P:(j + 1) * P],
                                    psS[:, s * P:(s + 1) * P],
                                    as1g[:, jt:jt + 1],
                                    cqb[:, tiles[s] * P:(tiles[s] + 1) * P],
                                    op0=ALU.mult, op1=ALU.mult,
                                )
                                if j == i_s:
                                    nc.vector.tensor_tensor(
                                        S1[s][:, j * P:(j + 1) * P],
                                        S1[s][:, j * P:(j + 1) * P],
                                        maskT[:], op=ALU.add
                                    )
                                    L = (i_s + 1) * P
                                    pt = s3.tile([P, (2 * s + 2) * P], BF16,
                                                 tag=f"PT{hl}_{s}",
                                                 name=f"PT{hl}_{s}", bufs=1)
                                    nc.scalar.activation(
                                        pt[:, 0:L], S1[s][:, 0:L], AF.Exp
                                    )
                                    PT[(s, hl)] = pt
                    for s in range(4):
                        t = tiles[s]
                        i_s = 2 * s + par
                        anat = s3.tile([P, 2 * P], F32, tag="anat", bufs=3)
                        for hl in range(2):
                            att = ps3a.tile([P, 129], F32, tag="att")
                            for j in range(i_s + 1):
                                nc.tensor.matmul(
                                    att[:],
                                    PT[(s, hl)][:, j * P:(j + 1) * P],
                                    vplus[b * 8 + j][:, hl * 129:
                                                     (hl + 1) * 129],
                                    start=(j == 0), stop=(j == i_s),
                                )
                            erec = s3.tile([P, 1], F32, tag="erec", bufs=3)
                            nc.vector.reciprocal(erec[:], att[:, 128:129])
                            nc.vector.tensor_scalar(
                                anat[:, hl * P:(hl + 1) * P],
                                att[:, 0:128], erec[:, 0:1], None,
                                op0=ALU.mult,
                            )
                        ao = s3.tile([P, 2 * P], BF16, tag="ao", bufs=3)
                        nc.vector.tensor_copy(ao[:], anat[:])
                        c = t // 2
                        nc.sync.dma_start(
                            a2a_in[par][c * P:(c + 1) * P, :], ao[:]
                        )

                def oproj_quant_h(h, sp):
                    """Load A2A result, per-row amax, quantize (no PE)."""
                    nc.scalar.dma_start(
                        x_o[h][:],
                        a2a_out[h].rearrange("(s r) f -> r s f", s=NCORES)
                    )
                    am = sp.tile([P, 1], F32, tag="am_o", bufs=2)
                    nc.vector.tensor_reduce(
                        am[:], x_o[h][:], op=ALU.max, axis=AX.X,
                        apply_absolute_value=True,
                    )
                    nc.vector.tensor_scalar(
                        asol[:, h:h + 1], am[:], 1e-8, None, op0=ALU.add
                    )
                    inva = sp.tile([P, 1], F32, tag="inva_o", bufs=2)
                    nc.vector.reciprocal(inva[:], asol[:, h:h + 1])
                    nc.vector.tensor_scalar(
                        qs_o[:, h:h + 1], inva[:], 127.0, None, op0=ALU.mult
                    )
                    nc.vector.tensor_scalar(
                        sc_ol[:, h:h + 1], asol[:, h:h + 1], cb[2][:, 0:1],
                        None, op0=ALU.mult
                    )
                    _quant(nc, sp, x_o[h][:], qs_o[:, h:h + 1], aq_o[h][:],
                           tag="qotmp")

                def oproj_pe_h(h, wo_strips, sp, ps4t, ps5o, sq_pool):
                    for kb in range(KT):
                        pst = ps4t.tile([P, P], BF16, tag="pst4")
                        nc.tensor.transpose(
                            pst[:], aq_o[h][:, kb * P:(kb + 1) * P], ident[:]
                        )
                        nc.vector.tensor_copy(
                            olhsT[h][:, kb * P:(kb + 1) * P], pst[:]
                        )
                    po = ps5o.tile([P, D], F32, tag="po")
                    for kb in range(KT):
                        for n in range(4):
                            nc.tensor.matmul(
                                po[:, n * 512:(n + 1) * 512],
                                olhsT[h][:, kb * P:(kb + 1) * P],
                                wo_strips[kb][:, n * 512:(n + 1) * 512],
                                start=(kb == 0), stop=(kb == KT - 1),
                            )
                    xr = sp.tile([P, D], F32, tag="xr", bufs=1)
                    nc.sync.dma_start(xr[:], x_rows[h * P:(h + 1) * P, :])
                    nc.vector.scalar_tensor_tensor(
                        x1t[h][:], po[:], sc_ol[:, h:h + 1], xr[:],
                        op0=ALU.mult, op1=ALU.add,
                    )
                    nc.sync.dma_start(x1_d[h * P:(h + 1) * P, :], x1t[h][:])
                    # rmsnorm2 + quant + transpose + stage AG2 chunk h
                    as_l2 = sp.tile([P, 1], F32, tag="as_l2", bufs=2)
                    aq2 = sp.tile([P, D], BF16, tag="aq2", bufs=2)
                    _rms_quant_rows(nc, sp, sq_pool, x1t[h], nw2,
                                    as_l2[:, 0:1], aq2[:])
                    nc.sync.dma_start(ag2s_in[h * P:(h + 1) * P, :], as_l2[:])
                    nc.gpsimd.collective_compute(
                        "AllGather", ALU.bypass, replica_groups=rg,
                        ins=[ag2s_in[h * P:(h + 1) * P, :].opt()],
                        outs=[ag2s_hout[h].opt()],
                    )
                    for kb in range(KT):
                        pst = ps4t.tile([P, P], BF16, tag="pst4")
                        nc.tensor.transpose(
                            pst[:], aq2[:, kb * P:(kb + 1) * P], ident[:]
                        )
                        aqs2 = sp.tile([P, P], BF16, tag="aqs2", bufs=3)
                        nc.vector.tensor_copy(aqs2[:], pst[:])
                        nc.sync.dma_start(
                            ag2c_in[h][kb * P:(kb + 1) * P, :], aqs2[:]
                        )
                    nc.gpsimd.collective_compute(
                        "AllGather", ALU.bypass, replica_groups=rg,
                        ins=[ag2c_in[h].opt()], outs=[ag2c_out[h].opt()],
                    )

                with tc.tile_pool(name="p5sc", bufs=2) as s5, \
                     tc.tile_pool(name="p3sc", bufs=4) as s3, \
                     tc.tile_pool(name="ps3s", bufs=2, space="PSUM") as ps3s, \
                     tc.tile_pool(name="ps3a", bufs=1, space="PSUM") as ps3a, \
                     tc.tile_pool(name="ps4t", bufs=1, space="PSUM") as ps4t, \
                     tc.tile_pool(name="ps5o", bufs=1, space="PSUM") as ps5o:
                    nwr2 = s5.tile([1, D], F32, tag="nwr2", bufs=1)
                    nc.scalar.dma_start(nwr2[:], norm2_w)
                    nc.gpsimd.partition_broadcast(nw2[:], nwr2[0:1, :])
                    sqp = s5.tile([P, D], F32, tag="sqp")
                    sq_pool = _FixedTilePool(sqp)
                    attn_quad(0, s3, ps3s, ps3a)
                    attn_quad(1, s3, ps3s, ps3a)
                    nc.gpsimd.collective_compute(
                        "AllToAll", ALU.bypass, replica_groups=rg,
                        ins=[a2a_in[0].opt()], outs=[a2a_out[0].opt()],
                    )
                    attn_quad(2, s3, ps3s, ps3a)
                    # prefetch o-proj weights for h=0 during attention
                    wo0 = []
                    for kb in range(KT):
                        wt = p4w.tile([P, D], BF16, tag="wo_t", bufs=6)
                        nc.sync.dma_start(wt[:], woT[kb * P:(kb + 1) * P, :])
                        wo0.append(wt)
                    oproj_quant_h(0, s5)
                    # o-proj h0 interleaved before quad3: fills quad3's
                    # ACT-bound waits and fires AG2 chunk 0 ~60us earlier
                    oproj_pe_h(0, wo0, s5, ps4t, ps5o, sq_pool)
                    attn_quad(3, s3, ps3s, ps3a)
                    nc.gpsimd.collective_compute(
                        "AllToAll", ALU.bypass, replica_groups=rg,
                        ins=[a2a_in[1].opt()], outs=[a2a_out[1].opt()],
                    )
                    oproj_quant_h(1, s5)
                    wo1 = []
                    for kb in range(KT):
                        wt = p4w.tile([P, D], BF16, tag="wo_t", bufs=6)
                        nc.sync.dma_start(wt[:], woT[kb * P:(kb + 1) * P, :])
                        wo1.append(wt)
                    oproj_pe_h(1, wo1, s5, ps4t, ps5o, sq_pool)
        # mega-pool 1 frees here

        # =========================================================
        # Mega-pool 2: MLP (phases 5,6)
        # =========================================================
        NG = 4
        NOC = 4
        OCW = D // NOC  # 512
        proc = [2 * c + h for h in range(LT) for c in range(NCORES)]
        with tc.tile_pool(name="mp2", bufs=1) as m2, \
             tc.tile_pool(name="mp2sc", bufs=2) as s6:
            wgu_sb = [m2.tile([P, 2 * OM], BF16, tag=f"wgu{k}", name=f"wgu{k}")
                      for k in range(KT)]
            for k in range(KT):
                nc.sync.dma_start(wgu_sb[k][:], wguT[k * P:(k + 1) * P, :])

            def load_scales_half(h):
                hs = slice(h * 8, (h + 1) * 8)
                nc.scalar.dma_start(
                    as2g[:, hs],
                    ag2s_hout[h].rearrange("(c p) o -> p (c o)", p=P)
                )
                nc.vector.tensor_scalar(
                    sc_g[:, hs], as2g[:, hs], cb[3][:, 0:1], None,
                    op0=ALU.mult
                )
                nc.vector.tensor_scalar(
                    sc_u[:, hs], as2g[:, hs], cb[4][:, 0:1], None,
                    op0=ALU.mult
                )

            load_scales_half(0)
            # one-hot row-select (host input): picks this core's own tile
            # column out of the AR'd scale tiles -> local down-proj scale,
            # replacing v2's ReduceScatter(max)
            rsel = m2.tile([P, NCORES], F32, tag="rsel")
            rselr = s6.tile([1, NCORES], F32, tag="rselr", bufs=1)
            nc.scalar.dma_start(rselr[:], rowsel_d)
            nc.gpsimd.partition_broadcast(rsel[:], rselr[0:1, :])

            asm_in = [dp.tile([4 * P, 1], F32, tag=f"asmi{g}", name=f"asmi{g}")
                      for g in range(NG)]
            asm_go = [dp.tile([4 * P, 1], F32, tag=f"asmo{g}", name=f"asmo{g}",
                              addr_space="Shared") for g in range(NG)]
            mT = [m2.tile([P, R], BF16, tag=f"mT{kb}", name=f"mT{kb}")
                  for kb in range(MT)]
            m_tiles = [m2.tile([P, OM], BF16, tag=f"m{t}", name=f"m{t}")
                       for t in range(RT)]
            a2t_src = [ag2c_out[h].rearrange(
                "(c k p) j -> c p k j", c=NCORES, k=KT, p=P)
                for h in range(LT)]
            x1r = [m2.tile([P, D], F32, tag=f"x1r{lt}", name=f"x1r{lt}")
                   for lt in range(LT)]
            for lt in range(LT):
                nc.sync.dma_start(x1r[lt][:], x1_d[lt * P:(lt + 1) * P, :])
            rs_in = [dp.tile([NCORES * P, OCW], BF16, tag=f"rsdi{i}",
                             name=f"rsdi{i}") for i in range(NOC * LT)]
            rs_out = [dp.tile([P, OCW], BF16, tag=f"rsdo{i}",
                              name=f"rsdo{i}") for i in range(NOC * LT)]

            def gate_up_compute(g, ps6):
                for tl in range(4):
                    pi = g * 4 + tl
                    t = proc[pi]
                    a2t = s6.tile([P, D], BF16, tag="a2t", bufs=2)
                    nc.sync.dma_start(a2t[:], a2t_src[t % 2][t // 2])
                    psg = []
                    for n in range(4):
                        pg = ps6.tile([P, 512], F32, tag="psg", bufs=4)
                        for kb in range(KT):
                            nc.tensor.matmul(
                                pg[:], a2t[:, kb * P:(kb + 1) * P],
                                wgu_sb[kb][:, n * 512:(n + 1) * 512],
                                start=(kb == 0), stop=(kb == KT - 1),
                            )
                        psg.append(pg)
                    for half in range(2):
                        # silu(g_deq) = g_deq * sigmoid(g_deq)
                        sig = s6.tile([P, 512], F32, tag="sig")
                        nc.scalar.activation(
                            sig[:], psg[half][:], AF.Sigmoid,
                            scale=sc_g[:, pi:pi + 1]
                        )
                        sgl = s6.tile([P, 512], F32, tag="sgl")
                        nc.vector.scalar_tensor_tensor(
                            sgl[:], psg[half][:], sc_g[:, pi:pi + 1],
                            sig[:], op0=ALU.mult, op1=ALU.mult,
                        )
                        nc.vector.scalar_tensor_tensor(
                            m_tiles[t][:, half * 512:(half + 1) * 512],
                            psg[2 + half][:], sc_u[:, pi:pi + 1], sgl[:],
                            op0=ALU.mult, op1=ALU.mult,
                        )
                    nc.vector.tensor_reduce(
                        asm[:, pi:pi + 1], m_tiles[t][:], op=ALU.max,
                        axis=AX.X, apply_absolute_value=True,
                    )
                gs = slice(g * 4, (g + 1) * 4)
                nc.gpsimd.dma_start(
                    asm_in[g].rearrange("(t p) o -> p (t o)", p=P),
                    asm[:, gs],
                )
                nc.gpsimd.collective_compute(
                    "AllReduce", ALU.max, replica_groups=rg,
                    ins=[asm_in[g].opt()], outs=[asm_go[g].opt()],
                )

            def post_ar(g, ps6):
                # emitted >=1 group after the AR fire, so none of these
                # queue instructions ever waits on an in-flight collective
                gs = slice(g * 4, (g + 1) * 4)
                nc.scalar.dma_start(
                    asmg[:, gs],
                    asm_go[g].rearrange("(t p) o -> p (t o)", p=P),
                )
                nc.vector.tensor_scalar(
                    asmg[:, gs], asmg[:, gs], 1e-8, None, op0=ALU.add
                )
                nc.vector.reciprocal(qsm[:, gs], asmg[:, gs])
                nc.vector.tensor_scalar(
                    qsm[:, gs], qsm[:, gs], 127.0, None, op0=ALU.mult
                )
                for tl in range(4):
                    pi = g * 4 + tl
                    t = proc[pi]
                    m_q = s6.tile([P, OM], BF16, tag="m_q", bufs=2)
                    _quant(nc, s6, m_tiles[t][:], qsm[:, pi:pi + 1],
                           m_q[:], tag="qtmp")
                    # PE transposes: DMA transposes (1.2us each) would
                    # occupy the sync/scalar queues ~40us per group and
                    # starve the a2t/wd loads behind them
                    for kb in range(MT):
                        pst = ps6.tile([P, P], BF16, tag="pst6", bufs=2)
                        nc.tensor.transpose(
                            pst[:], m_q[:, kb * P:(kb + 1) * P], ident[:]
                        )
                        nc.vector.tensor_copy(
                            mT[kb][:, t * P:(t + 1) * P], pst[:]
                        )

            def sc_dl_half(lt):
                # sc_dl[:, lt] = asmg[pi of own tile 2c+lt] * wsd/127 via
                # one-hot mask (columns lt*8..lt*8+8 are all written by now)
                tmpm = s6.tile([P, NCORES], F32, tag="sctmp", bufs=1)
                nc.vector.tensor_tensor(
                    tmpm[:], asmg[:, lt * 8:(lt + 1) * 8], rsel[:],
                    op=ALU.mult
                )
                scr = s6.tile([P, 1], F32, tag="scr", bufs=1)
                nc.vector.tensor_reduce(
                    scr[:], tmpm[:], op=ALU.max, axis=AX.X
                )
                nc.vector.tensor_scalar(
                    sc_dl[:, lt:lt + 1], scr[:], cb[5][:, 0:1], None,
                    op0=ALU.mult
                )

            def down_pass(par, p7w, ps6):
                for oc in range(NOC):
                    wd_oc = p7w.tile([P, MT * OCW], BF16, tag="wd_oc")
                    for kb in range(MT):
                        nc.sync.dma_start(
                            wd_oc[:, kb * OCW:(kb + 1) * OCW],
                            wdT[kb * P:(kb + 1) * P,
                                oc * OCW:(oc + 1) * OCW],
                        )
                    ci = oc * 2 + par
                    for c in range(NCORES):
                        t = 2 * c + par
                        pso = ps6.tile([P, OCW], F32, tag="dps", bufs=2)
                        for kb in range(MT):
                            nc.tensor.matmul(
                                pso[:], mT[kb][:, t * P:(t + 1) * P],
                                wd_oc[:, kb * OCW:(kb + 1) * OCW],
                                start=(kb == 0), stop=(kb == MT - 1),
                            )
                        # bf16 RS halves wire traffic; partials are ints
                        # < 2^18 so bf16 adds ~2^-9 relative rounding
                        dsb = s6.tile([P, OCW], BF16, tag="dsb", bufs=3)
                        nc.vector.tensor_copy(dsb[:], pso[:])
                        nc.sync.dma_start(
                            rs_in[ci][c * P:(c + 1) * P, :], dsb[:]
                        )
                    nc.gpsimd.collective_compute(
                        "ReduceScatter", ALU.add, replica_groups=rg,
                        ins=[rs_in[ci].opt()], outs=[rs_out[ci].opt()],
                    )

            def down_drain(par):
                for oc in range(NOC):
                    ci = oc * 2 + par
                    ysb = s6.tile([P, OCW], BF16, tag="ysb")
                    nc.scalar.dma_start(ysb[:], rs_out[ci][:])
                    ot = s6.tile([P, OCW], F32, tag="ot")
                    nc.vector.scalar_tensor_tensor(
                        ot[:], ysb[:], sc_dl[:, par:par + 1],
                        x1r[par][:, oc * OCW:(oc + 1) * OCW],
                        op0=ALU.mult, op1=ALU.add,
                    )
                    nc.sync.dma_start(
                        out_d[par * P:(par + 1) * P,
                              oc * OCW:(oc + 1) * OCW],
                        ot[:],
                    )

            with tc.tile_pool(name="ps6", bufs=1, space="PSUM") as ps6, \
                 tc.tile_pool(name="p7w", bufs=2) as p7w:
                gate_up_compute(0, ps6)
                gate_up_compute(1, ps6)
                post_ar(0, ps6)
                load_scales_half(1)
                gate_up_compute(2, ps6)
                post_ar(1, ps6)
                sc_dl_half(0)
                # down-proj even row tiles: their mT strips (groups 0-1)
                # are transposed by now; the 4 RS-evens overlap group 3
                down_pass(0, p7w, ps6)
                post_ar(2, ps6)
                gate_up_compute(3, ps6)
                post_ar(3, ps6)
                sc_dl_half(1)
                down_drain(0)
                down_pass(1, p7w, ps6)
                down_drain(1)

    nc.compile()
    return nc


def _ternary_quant(w):
    """BitNet weight quant on host: ternary bf16 (exact) + f32 scale."""
    ws = float(np.abs(np.asarray(w, np.float64)).mean()) + 1e-8
    q = np.clip(np.round(np.asarray(w, np.float64) / ws), -1.0, 1.0)
    return q.astype(ml_dtypes.bfloat16), np.float32(ws)


def _prep_in_maps(inputs):
    x = np.asarray(inputs["x"], np.float32).reshape(R, D)
    wq, wsq = _ternary_quant(inputs["wq"])
    wk, wsk = _ternary_quant(inputs["wk"])
    wv, wsv = _ternary_quant(inputs["wv"])
    wo, wso = _ternary_quant(inputs["wo"])
    wg, wsg = _ternary_quant(inputs["wg"])
    wu, wsu = _ternary_quant(inputs["wu"])
    wd, wsd = _ternary_quant(inputs["wd"])
    n1 = np.asarray(inputs["norm1_w"], np.float32).reshape(1, D)
    n2 = np.asarray(inputs["norm2_w"], np.float32).reshape(1, D)

    ident = np.eye(P, dtype=ml_dtypes.bfloat16)
    iv, jv = np.mgrid[0:P, 0:P]
    # transposed causal block mask: key row u > query col v is masked
    causal_t = np.where(iv <= jv, 0.0, -1e30).astype(np.float32)
    wconsts = np.array([[
        wsq * wsk * INV_SQRT_HD / (127.0 * 127.0),
        wsv / 127.0, wso / 127.0, wsg / 127.0, wsu / 127.0, wsd / 127.0,
        0.0, 0.0,
    ]], np.float32)
    woT_full = np.ascontiguousarray(wo.T)

    in_maps = []
    for c in range(NCORES):
        qs = slice(c * OQ, (c + 1) * OQ)
        ms = slice(c * OM, (c + 1) * OM)
        # per-head-major qkv columns: q-h0, q-h1, k-h0, k-h1, v-h0, v-h1
        rowsel = np.zeros((1, NCORES), np.float32)
        rowsel[0, c] = 1.0
        in_maps.append({
            "x_rows": np.ascontiguousarray(x[c * RL:(c + 1) * RL]),
            "rowsel": rowsel,
            "wqkvT": np.ascontiguousarray(
                np.concatenate([wq[qs], wk[qs], wv[qs]], 0).T
            ),
            "woT": woT_full,
            "wguT": np.ascontiguousarray(
                np.concatenate([wg[ms], wu[ms]], 0).T
            ),
            "wdT": np.ascontiguousarray(wd[:, ms].T),
            "norm1_w": n1,
            "norm2_w": n2,
            "ident_b": ident,
            "causal_t": causal_t,
            "wconsts": wconsts,
        })
    return in_maps


def kernel(**inputs) -> np.ndarray:
    global _CACHED_NC
    if _CACHED_NC is None:
        _CACHED_NC = build_program()
    nc = _CACHED_NC
    in_maps = _prep_in_maps(inputs)
    res = run_bass_kernel_spmd(nc, in_maps, core_ids=list(range(NCORES)))
    out = np.concatenate([res.results[c]["out"] for c in range(NCORES)], 0)
    return out.reshape(B, S, D).astype(np.float32)


# revision 36
# speedup vs baseline: 1.1606x; 1.0061x over previous
"""BitNet transformer block on 8 Trainium2 NeuronCores (Megatron tensor-parallel).

Self-contained: builds one SPMD Bass/Tile program, shards inputs on host,
runs via run_bass_kernel_spmd, gathers the output.

v3 design (vs v2): eliminate tensor-engine idle (v2 trace: PE union-busy
560us of 1045us span; ~440us of idle gaps around collectives).
  - AG1/AG2 of activations split into two 512KB column chunks (one per local
    row tile); QKV / gate-up process row-chunks as they arrive.
  - Attention scores batched: 4 query tiles per matmul (N=512) against each
    key tile. Query tiles live in qkT in chunk-slot order (evens then odds)
    so quads are contiguous.
  - Attention outputs are AllToAll'd RAW (bf16, natural [rows, feats]
    layout) in two chunks fired mid-attention; the int8-grid quantization
    for o-proj happens AFTER the A2A where each core owns all features of
    its rows, so the per-row a_scale is local. This removes v2's four
    2KB AllReduce(max) ops (12-25us each) and the 8KB ReduceScatter.
  - o-proj / x1 / rmsnorm2 / AG2 pipelined per row tile.
  - MLP: m tiles kept in bf16 so all 16 stay resident (no buffer
    starvation around the scale AllReduces); gate/up uses [P,512] PSUM
    chunks; down-proj ReduceScatter split per (oc, even/odd row tiles)
    into 8 x 1MB chunks so the exposed tail is one 1MB RS.

Numerics: quantized activations (ints in [-127,127]) and ternary weights are
exact in bf16; matmuls accumulate in fp32 PSUM, so every BitNet matmul is
exact integer arithmetic. Attention outputs and m cross the wire / live in
bf16 before their quantization: this adds <=0.25 int-step of extra rounding
on top of the inherent 0.5-step quant noise. Rounding uses the fp32
magic-constant trick (+1.5*2^23) matching jnp.round ties-to-even.
"""

import os

import numpy as np
import ml_dtypes

import concourse.bacc as bacc
import concourse.mybir as mybir
import concourse.tile as tile
from concourse.bass_utils import run_bass_kernel_spmd

F32 = mybir.dt.float32
BF16 = mybir.dt.bfloat16
AF = mybir.ActivationFunctionType
ALU = mybir.AluOpType
AX = mybir.AxisListType

NCORES = 8
B, S, D, H, MLP = 2, 1024, 2048, 16, 8192
HD = 128
R = B * S                 # 2048 rows total
RL = R // NCORES          # 256 rows per core (row shard)
OQ = D // NCORES          # 256 qkv out cols per core (2 heads)
OM = MLP // NCORES        # 1024 mlp cols per core
P = 128
KT = D // P               # 16 feature chunks
RT = R // P               # 16 row tiles
LT = RL // P              # 2 local row tiles
ST = S // P               # 8 seq tiles per batch
MT = OM // P              # 8 mlp k-chunks per core
MAGIC = 12582912.0        # 1.5 * 2**23: fp32 round-to-nearest-even magic
INV_SQRT_HD = 1.0 / float(np.sqrt(HD))

_CACHED_NC = None


def SLOT(t):
    """qkT column slot for global row tile t (evens first, then odds)."""
    return (t % 2) * 8 + t // 2


def _quant(nc, sp, src_ap, qscale_ap, out_bf_ap, tag="qtmp"):
    """out_bf = round(src * qscale) as bf16.

    fp32 +MAGIC rounds to integer (RNE); ACT subtracts MAGIC and casts to
    bf16 (small ints are exact in bf16).
    """
    F = src_ap.shape[1]
    CH = min(F, 1024)  # bound the fp32 scratch to 4KB/partition
    for c0 in range(0, F, CH):
        tmp = sp.tile([src_ap.shape[0], CH], F32, tag=tag, name=tag)
        nc.vector.tensor_scalar(
            tmp[:], src_ap[:, c0:c0 + CH], qscale_ap, MAGIC,
            op0=ALU.mult, op1=ALU.add,
        )
        nc.scalar.activation(
            out_bf_ap[:, c0:c0 + CH], tmp[:], AF.Copy, bias=-MAGIC, scale=1.0
        )


class _FixedTilePool:
    """Adapter handing out a pre-allocated tile (for _rms_quant_rows sqd)."""

    def __init__(self, t):
        self._t = t

    def tile(self, shape, dtype, tag=""):
        return self._t


def _rms_quant_rows(nc, sp, ps_dummy, src_tile, nw_tile, as_out_ap, aq_out_ap):
    """rmsnorm + abs-max + int8-grid quantize for one [128, D] row tile.

    Writes a_scale (max|h|+1e-8, h = src/rms*nw) to as_out_ap [128,1] and
    the quantized bf16 ints to aq_out_ap. Mutates src_tile in place
    (src *= nw).
    """
    sqd = ps_dummy.tile([P, D], F32, tag="sqd")
    ssq = sp.tile([P, 1], F32, tag="ssq")
    nc.scalar.activation(sqd[:], src_tile[:], AF.Square, accum_out=ssq[:])
    rms = sp.tile([P, 1], F32, tag="rms")
    nc.vector.tensor_scalar(
        rms[:], ssq[:], 1.0 / D, 1e-6, op0=ALU.mult, op1=ALU.add
    )
    nc.scalar.activation(rms[:], rms[:], AF.Sqrt)
    rinv = sp.tile([P, 1], F32, tag="rinv")
    nc.vector.reciprocal(rinv[:], rms[:])
    nc.vector.tensor_tensor(src_tile[:], src_tile[:], nw_tile[:], op=ALU.mult)
    amax = sp.tile([P, 1], F32, tag="amax")
    nc.vector.tensor_reduce(
        amax[:], src_tile[:], op=ALU.max, axis=AX.X, apply_absolute_value=True
    )
    nc.vector.tensor_scalar(
        as_out_ap, amax[:], rinv[:], 1e-8, op0=ALU.mult, op1=ALU.add
    )
    inva = sp.tile([P, 1], F32, tag="inva")
    nc.vector.reciprocal(inva[:], as_out_ap)
    qs = sp.tile([P, 1], F32, tag="qs")
    nc.vector.tensor_scalar(
        qs[:], inva[:], rinv[:], 127.0, op0=ALU.mult, op1=ALU.mult
    )
    _quant(nc, sp, src_tile[:], qs[:, 0:1], aq_out_ap)


def build_program():
    nc = bacc.Bacc(
        "TRN2",
        target_bir_lowering=False,
        debug=False,
        enable_asserts=True,
        num_devices=NCORES,
    )
    rg = [list(range(NCORES))]

    # ---------------- I/O (identical layouts to v2) ----------------
    x_rows = nc.dram_tensor("x_rows", [RL, D], F32, kind="ExternalInput").ap()
    wqkvT = nc.dram_tensor("wqkvT", [D, 3 * OQ], BF16, kind="ExternalInput").ap()
    woT = nc.dram_tensor("woT", [D, D], BF16, kind="ExternalInput").ap()
    wguT = nc.dram_tensor("wguT", [D, 2 * OM], BF16, kind="ExternalInput").ap()
    wdT = nc.dram_tensor("wdT", [OM, D], BF16, kind="ExternalInput").ap()
    norm1_w = nc.dram_tensor("norm1_w", [1, D], F32, kind="ExternalInput").ap()
    norm2_w = nc.dram_tensor("norm2_w", [1, D], F32, kind="ExternalInput").ap()
    ident_b = nc.dram_tensor("ident_b", [P, P], BF16, kind="ExternalInput").ap()
    causal_t = nc.dram_tensor("causal_t", [P, P], F32, kind="ExternalInput").ap()
    wconsts = nc.dram_tensor("wconsts", [1, 8], F32, kind="ExternalInput").ap()
    rowsel_d = nc.dram_tensor("rowsel", [1, NCORES], F32,
                              kind="ExternalInput").ap()
    out_d = nc.dram_tensor("out", [RL, D], F32, kind="ExternalOutput").ap()

    with tile.TileContext(nc) as tc, \
         tc.tile_pool(name="persist", bufs=1) as pp, \
         tc.tile_pool(name="dram", bufs=1, space="DRAM") as dp:

        # ---------------- constants ----------------
        ident = pp.tile([P, P], BF16, tag="ident")
        nc.sync.dma_start(ident[:], ident_b)
        maskT = pp.tile([P, P], F32, tag="maskT")
        nc.sync.dma_start(maskT[:], causal_t)
        wcrow = pp.tile([1, 8], F32, tag="wcrow")
        nc.sync.dma_start(wcrow[:], wconsts)
        # 0: wsq*wsk/(127^2 sqrt(HD)), 1: wsv/127, 2: wso/127,
        # 3: wsg/127, 4: wsu/127, 5: wsd/127
        cb = {}
        for slot in range(6):
            cb[slot] = pp.tile([P, 1], F32, tag=f"cb{slot}", name=f"cb{slot}")
            nc.gpsimd.partition_broadcast(
                cb[slot][:], wcrow[0:1, slot:slot + 1]
            )

        # persistent scale tiles
        as1g = pp.tile([P, RT], F32, tag="as1g")
        scv = pp.tile([P, RT], F32, tag="scv")
        asol = pp.tile([P, LT], F32, tag="asol")
        sc_ol = pp.tile([P, LT], F32, tag="sc_ol")
        qs_o = pp.tile([P, LT], F32, tag="qs_o")
        as2g = pp.tile([P, RT], F32, tag="as2g")
        sc_g = pp.tile([P, RT], F32, tag="sc_g")
        sc_u = pp.tile([P, RT], F32, tag="sc_u")
        asm = pp.tile([P, RT], F32, tag="asm")      # proc-order columns
        asmg = pp.tile([P, RT], F32, tag="asmg")
        qsm = pp.tile([P, RT], F32, tag="qsm")
        asml = pp.tile([P, LT], F32, tag="asml")
        sc_dl = pp.tile([P, LT], F32, tag="sc_dl")

        # collective DRAM buffers
        ag1c_in = [dp.tile([D, P], BF16, tag=f"ag1ci{h}", name=f"ag1ci{h}")
                   for h in range(LT)]
        ag1c_out = [dp.tile([NCORES * D, P], BF16, tag=f"ag1co{h}",
                            name=f"ag1co{h}", addr_space="Shared")
                    for h in range(LT)]
        ag1s_in = dp.tile([RL, 1], F32, tag="ag1s_in")
        ag1s_out = dp.tile([R, 1], F32, tag="ag1s_out", addr_space="Shared")
        a2a_in = [dp.tile([NCORES * P, 2 * P], BF16, tag=f"a2ai{h}",
                          name=f"a2ai{h}") for h in range(LT)]
        a2a_out = [dp.tile([NCORES * P, 2 * P], BF16, tag=f"a2ao{h}",
                           name=f"a2ao{h}") for h in range(LT)]
        x1_d = dp.tile([RL, D], F32, tag="x1_d")
        ag2c_in = [dp.tile([D, P], BF16, tag=f"ag2ci{h}", name=f"ag2ci{h}")
                   for h in range(LT)]
        ag2c_out = [dp.tile([NCORES * D, P], BF16, tag=f"ag2co{h}",
                            name=f"ag2co{h}", addr_space="Shared")
                    for h in range(LT)]
        ag2s_in = dp.tile([RL, 1], F32, tag="ag2s_in")
        ag2s_hout = [dp.tile([NCORES * P, 1], F32, tag=f"ag2sh{h}",
                             name=f"ag2sh{h}", addr_space="Shared")
                     for h in range(LT)]

        # =========================================================
        # Mega-pool 1: phases 1,2,3 + o-proj + phase 4
        # =========================================================
        with tc.tile_pool(name="mp1", bufs=1) as m1:
            nw1 = m1.tile([P, D], F32, tag="nw1")
            cqb = m1.tile([P, R], F32, tag="cqb")
            qkT = [m1.tile([P, R], BF16, tag=f"qkT{ch}", name=f"qkT{ch}")
                   for ch in range(4)]
            vplus = [m1.tile([P, 258], BF16, tag=f"vp{t}", name=f"vp{t}")
                     for t in range(RT)]
            x1t = [m1.tile([P, D], F32, tag=f"x1t{lt}", name=f"x1t{lt}")
                   for lt in range(LT)]

            # ---- Phase 1: local rmsnorm1 + quant + transpose + chunked AG
            with tc.tile_pool(name="p2w", bufs=1) as p2m:
                wqkv_sb = [p2m.tile([P, 3 * OQ], BF16, tag=f"wqkv{k}",
                                    name=f"wqkv{k}") for k in range(KT)]
                with tc.tile_pool(name="p1sc", bufs=2) as s1, \
                     tc.tile_pool(name="ps1", bufs=1, space="PSUM") as ps1, \
                     tc.tile_pool(name="ps1t", bufs=2, space="PSUM") as ps1t:
                    # pre-emit x loads (sync); weight prefetch on scalar queue
                    xt = [s1.tile([P, D], F32, tag=f"xt{lt}", name=f"xt{lt}",
                                  bufs=1) for lt in range(LT)]
                    for lt in range(LT):
                        nc.sync.dma_start(
                            xt[lt][:], x_rows[lt * P:(lt + 1) * P, :]
                        )
                    for k in range(KT):
                        nc.scalar.dma_start(
                            wqkv_sb[k][:], wqkvT[k * P:(k + 1) * P, :]
                        )
                    nwr = s1.tile([1, D], F32, tag="nwr", bufs=1)
                    nc.sync.dma_start(nwr[:], norm1_w)
                    nc.gpsimd.partition_broadcast(nw1[:], nwr[0:1, :])
                    for lt in range(LT):
                        as_l = s1.tile([P, 1], F32, tag="as_l")
                        aq = s1.tile([P, D], BF16, tag="aq", bufs=1)
                        _rms_quant_rows(nc, s1, ps1, xt[lt], nw1,
                                        as_l[:, 0:1], aq[:])
                        nc.sync.dma_start(
                            ag1s_in[lt * P:(lt + 1) * P, :], as_l[:]
                        )
                        for kb in range(KT):
                            pst = ps1t.tile([P, P], BF16, tag="pst")
                            nc.tensor.transpose(
                                pst[:], aq[:, kb * P:(kb + 1) * P], ident[:]
                            )
                            aqs = s1.tile([P, P], BF16, tag="aqs", bufs=3)
                            nc.vector.tensor_copy(aqs[:], pst[:])
                            nc.sync.dma_start(
                                ag1c_in[lt][kb * P:(kb + 1) * P, :], aqs[:]
                            )
                        if lt == 1:
                            # scales AG between the two act chunks (its input
                            # is complete once both as_l DMAs have landed)
                            nc.gpsimd.collective_compute(
                                "AllGather", ALU.bypass, replica_groups=rg,
                                ins=[ag1s_in.opt()], outs=[ag1s_out.opt()],
                            )
                        nc.gpsimd.collective_compute(
                            "AllGather", ALU.bypass, replica_groups=rg,
                            ins=[ag1c_in[lt].opt()], outs=[ag1c_out[lt].opt()],
                        )

                # ---- Phase 2: QKV per AG chunk ----
                # qkT[ch] columns in chunk-slot order: tile t at SLOT(t)*P
                with tc.tile_pool(name="ps2qk", bufs=2, space="PSUM") as ps2qk, \
                     tc.tile_pool(name="ps2v", bufs=2, space="PSUM") as ps2v:
                    for h in range(LT):
                        a1T = [p2m.tile([P, NCORES * P], BF16,
                                        tag=f"a1T{kb}", name=f"a1T{kb}",
                                        bufs=2) for kb in range(KT)]
                        src = ag1c_out[h].rearrange(
                            "(c k p) j -> k p c j", c=NCORES, k=KT, p=P
                        )
                        for kb in range(KT):
                            nc.sync.dma_start(a1T[kb][:], src[kb])
                        for ch in range(4):
                            psq = ps2qk.tile([P, NCORES * P], F32, tag="psq")
                            for kb in range(KT):
                                for g in range(2):
                                    nc.tensor.matmul(
                                        psq[:, g * 512:(g + 1) * 512],
                                        wqkv_sb[kb][:, ch * P:(ch + 1) * P],
                                        a1T[kb][:, g * 512:(g + 1) * 512],
                                        start=(kb == 0), stop=(kb == KT - 1),
                                    )
                            nc.vector.tensor_copy(
                                qkT[ch][:, h * NCORES * P:
                                        (h + 1) * NCORES * P], psq[:]
                            )
                        if h == 0:
                            # scale prep off the sync queue (scalar DMAs)
                            nc.scalar.dma_start(
                                as1g[:],
                                ag1s_out.rearrange("(t p) o -> p (t o)", p=P)
                            )
                            as1row = p2m.tile([1, R], F32, tag="as1row")
                            nc.scalar.dma_start(
                                as1row[:], ag1s_out.rearrange("r o -> o r")
                            )
                            nc.vector.tensor_scalar(
                                as1row[:], as1row[:], cb[0][0:1, 0:1], None,
                                op0=ALU.mult
                            )
                            nc.gpsimd.partition_broadcast(cqb[:],
                                                          as1row[0:1, :])
                            nc.vector.tensor_scalar(
                                scv[:], as1g[:], cb[1][:, 0:1], None,
                                op0=ALU.mult
                            )
                        for c in range(NCORES):
                            t = 2 * c + h
                            psv = ps2v.tile([P, 2 * P], F32, tag="psv")
                            for kb in range(KT):
                                nc.tensor.matmul(
                                    psv[:], a1T[kb][:, c * P:(c + 1) * P],
                                    wqkv_sb[kb][:, 512:768],
                                    start=(kb == 0), stop=(kb == KT - 1),
                                )
                            nc.vector.tensor_scalar(
                                vplus[t][:, 0:128], psv[:, 0:128],
                                scv[:, t:t + 1], None, op0=ALU.mult,
                            )
                            nc.vector.tensor_scalar(
                                vplus[t][:, 129:257], psv[:, 128:256],
                                scv[:, t:t + 1], None, op0=ALU.mult,
                            )
                            nc.vector.memset(vplus[t][:, 128:129], 1.0)
                            nc.vector.memset(vplus[t][:, 257:258], 1.0)

            # ---- Phase 3: attention (quads) + o-proj + phase 4 ----
            with tc.tile_pool(name="p4w", bufs=1) as p4w, \
                 tc.tile_pool(name="p3x", bufs=1) as p3x:
                x_o = [p3x.tile([P, D], BF16, tag=f"xo{h}", name=f"xo{h}")
                       for h in range(LT)]
                aq_o = [p3x.tile([P, D], BF16, tag=f"aqo{h}", name=f"aqo{h}")
                        for h in range(LT)]
                olhsT = [p3x.tile([P, D], BF16, tag=f"olhsT{h}",
                                  name=f"olhsT{h}") for h in range(LT)]
                nw2 = p3x.tile([P, D], F32, tag="nw2")

                def attn_quad(q, s3, ps3s, ps3a):
                    b = [0, 1, 0, 1][q]
                    par = q // 2
                    imax = 6 + par
                    tiles = [b * 8 + 2 * s + par for s in range(4)]
                    qc0 = q * 4 * P
                    PT = {}
                    for hl in range(2):
                        S1 = [s3.tile([P, (2 * s + 2) * P], F32,
                                      tag=f"S1_{s}", name=f"S1_{s}", bufs=2)
                              for s in range(4)]
                        for j in range(imax + 1):
                            jt = b * 8 + j
                            psS = ps3s.tile([P, 4 * P], F32, tag="psS")
                            nc.tensor.matmul(
                                psS[:],
                                qkT[2 + hl][:, SLOT(jt) * P:(SLOT(jt) + 1) * P],
                                qkT[hl][:, qc0:qc0 + 4 * P],
                                start=True, stop=True,
                            )
                            for s in range(4):
                                i_s = 2 * s + par
                                if j > i_s:
                                    continue
                                nc.vector.scalar_tensor_tensor(
                                    S1[s][:, j * P:(j + 1) * P],
                                    psS[:, s * P:(s + 1) * P],
                                    as1g[:, jt:jt + 1],
                                    cqb[:, tiles[s] * P:(tiles[s] + 1) * P],
                                    op0=ALU.mult, op1=ALU.mult,
                                )
                                if j == i_s:
                                    nc.vector.tensor_tensor(
                                        S1[s][:, j * P:(j + 1) * P],
                                        S1[s][:, j * P:(j + 1) * P],
                                        maskT[:], op=ALU.add
                                    )
                                    L = (i_s + 1) * P
                                    pt = s3.tile([P, (2 * s + 2) * P], BF16,
                                                 tag=f"PT{hl}_{s}",
                                                 name=f"PT{hl}_{s}", bufs=1)
                                    nc.scalar.activation(
                                        pt[:, 0:L], S1[s][:, 0:L], AF.Exp
                                    )
                                    PT[(s, hl)] = pt
                    for s in range(4):
                        t = tiles[s]
                        i_s = 2 * s + par
                        anat = s3.tile([P, 2 * P], F32, tag="anat", bufs=3)
                        for hl in range(2):
                            att = ps3a.tile([P, 129], F32, tag="att")
                            for j in range(i_s + 1):
                                nc.tensor.matmul(
                                    att[:],
                                    PT[(s, hl)][:, j * P:(j + 1) * P],
                                    vplus[b * 8 + j][:, hl * 129:
                                                     (hl + 1) * 129],
                                    start=(j == 0), stop=(j == i_s),
                                )
                            erec = s3.tile([P, 1], F32, tag="erec", bufs=3)
                            nc.vector.reciprocal(erec[:], att[:, 128:129])
                            nc.vector.tensor_scalar(
                                anat[:, hl * P:(hl + 1) * P],
                                att[:, 0:128], erec[:, 0:1], None,
                                op0=ALU.mult,
                            )
                        ao = s3.tile([P, 2 * P], BF16, tag="ao", bufs=3)
                        nc.vector.tensor_copy(ao[:], anat[:])
                        c = t // 2
                        nc.sync.dma_start(
                            a2a_in[par][c * P:(c + 1) * P, :], ao[:]
                        )

                def oproj_quant_h(h, sp):
                    """Load A2A result, per-row amax, quantize (no PE)."""
                    nc.scalar.dma_start(
                        x_o[h][:],
                        a2a_out[h].rearrange("(s r) f -> r s f", s=NCORES)
                    )
                    am = sp.tile([P, 1], F32, tag="am_o", bufs=2)
                    nc.vector.tensor_reduce(
                        am[:], x_o[h][:], op=ALU.max, axis=AX.X,
                        apply_absolute_value=True,
                    )
                    nc.vector.tensor_scalar(
                        asol[:, h:h + 1], am[:], 1e-8, None, op0=ALU.add
                    )
                    inva = sp.tile([P, 1], F32, tag="inva_o", bufs=2)
                    nc.vector.reciprocal(inva[:], asol[:, h:h + 1])
                    nc.vector.tensor_scalar(
                        qs_o[:, h:h + 1], inva[:], 127.0, None, op0=ALU.mult
                    )
                    nc.vector.tensor_scalar(
                        sc_ol[:, h:h + 1], asol[:, h:h + 1], cb[2][:, 0:1],
                        None, op0=ALU.mult
                    )
                    _quant(nc, sp, x_o[h][:], qs_o[:, h:h + 1], aq_o[h][:],
                           tag="qotmp")

                def oproj_pe_h(h, wo_strips, sp, ps4t, ps5o, sq_pool):
                    for kb in range(KT):
                        pst = ps4t.tile([P, P], BF16, tag="pst4")
                        nc.tensor.transpose(
                            pst[:], aq_o[h][:, kb * P:(kb + 1) * P], ident[:]
                        )
                        nc.vector.tensor_copy(
                            olhsT[h][:, kb * P:(kb + 1) * P], pst[:]
                        )
                    po = ps5o.tile([P, D], F32, tag="po")
                    for kb in range(KT):
                        for n in range(4):
                            nc.tensor.matmul(
                                po[:, n * 512:(n + 1) * 512],
                                olhsT[h][:, kb * P:(kb + 1) * P],
                                wo_strips[kb][:, n * 512:(n + 1) * 512],
                                start=(kb == 0), stop=(kb == KT - 1),
                            )
                    xr = sp.tile([P, D], F32, tag="xr", bufs=1)
                    nc.sync.dma_start(xr[:], x_rows[h * P:(h + 1) * P, :])
                    nc.vector.scalar_tensor_tensor(
                        x1t[h][:], po[:], sc_ol[:, h:h + 1], xr[:],
                        op0=ALU.mult, op1=ALU.add,
                    )
                    nc.sync.dma_start(x1_d[h * P:(h + 1) * P, :], x1t[h][:])
                    # rmsnorm2 + quant + transpose + stage AG2 chunk h
                    as_l2 = sp.tile([P, 1], F32, tag="as_l2", bufs=2)
                    aq2 = sp.tile([P, D], BF16, tag="aq2", bufs=2)
                    _rms_quant_rows(nc, sp, sq_pool, x1t[h], nw2,
                                    as_l2[:, 0:1], aq2[:])
                    nc.sync.dma_start(ag2s_in[h * P:(h + 1) * P, :], as_l2[:])
                    for kb in range(KT):
                        pst = ps4t.tile([P, P], BF16, tag="pst4")
                        nc.tensor.transpose(
                            pst[:], aq2[:, kb * P:(kb + 1) * P], ident[:]
                        )
                        aqs2 = sp.tile([P, P], BF16, tag="aqs2", bufs=3)
                        nc.vector.tensor_copy(aqs2[:], pst[:])
                        nc.sync.dma_start(
                            ag2c_in[h][kb * P:(kb + 1) * P, :], aqs2[:]
                        )
                    nc.gpsimd.collective_compute(
                        "AllGather", ALU.bypass, replica_groups=rg,
                        ins=[ag2c_in[h].opt()], outs=[ag2c_out[h].opt()],
                    )
                    # scales AG after the act chunk: drains need it later
                    # than the matmuls need the act chunk
                    nc.gpsimd.collective_compute(
                        "AllGather", ALU.bypass, replica_groups=rg,
                        ins=[ag2s_in[h * P:(h + 1) * P, :].opt()],
                        outs=[ag2s_hout[h].opt()],
                    )

                with tc.tile_pool(name="p5sc", bufs=2) as s5, \
                     tc.tile_pool(name="p3sc", bufs=4) as s3, \
                     tc.tile_pool(name="ps3s", bufs=2, space="PSUM") as ps3s, \
                     tc.tile_pool(name="ps3a", bufs=1, space="PSUM") as ps3a, \
                     tc.tile_pool(name="ps4t", bufs=1, space="PSUM") as ps4t, \
                     tc.tile_pool(name="ps5o", bufs=1, space="PSUM") as ps5o:
                    nwr2 = s5.tile([1, D], F32, tag="nwr2", bufs=1)
                    nc.scalar.dma_start(nwr2[:], norm2_w)
                    nc.gpsimd.partition_broadcast(nw2[:], nwr2[0:1, :])
                    sqp = s5.tile([P, D], F32, tag="sqp")
                    sq_pool = _FixedTilePool(sqp)
                    attn_quad(0, s3, ps3s, ps3a)
                    attn_quad(1, s3, ps3s, ps3a)
                    nc.gpsimd.collective_compute(
                        "AllToAll", ALU.bypass, replica_groups=rg,
                        ins=[a2a_in[0].opt()], outs=[a2a_out[0].opt()],
                    )
                    attn_quad(2, s3, ps3s, ps3a)
                    # prefetch o-proj weights for h=0 during attention
                    wo0 = []
                    for kb in range(KT):
                        wt = p4w.tile([P, D], BF16, tag="wo_t", bufs=6)
                        nc.sync.dma_start(wt[:], woT[kb * P:(kb + 1) * P, :])
                        wo0.append(wt)
                    oproj_quant_h(0, s5)
                    # o-proj h0 interleaved before quad3: fills quad3's
                    # ACT-bound waits and fires AG2 chunk 0 ~60us earlier
                    oproj_pe_h(0, wo0, s5, ps4t, ps5o, sq_pool)
                    attn_quad(3, s3, ps3s, ps3a)
                    nc.gpsimd.collective_compute(
                        "AllToAll", ALU.bypass, replica_groups=rg,
                        ins=[a2a_in[1].opt()], outs=[a2a_out[1].opt()],
                    )
                    oproj_quant_h(1, s5)
                    wo1 = []
                    for kb in range(KT):
                        wt = p4w.tile([P, D], BF16, tag="wo_t", bufs=6)
                        nc.sync.dma_start(wt[:], woT[kb * P:(kb + 1) * P, :])
                        wo1.append(wt)
                    oproj_pe_h(1, wo1, s5, ps4t, ps5o, sq_pool)
        # mega-pool 1 frees here

        # =========================================================
        # Mega-pool 2: MLP (phases 5,6)
        # =========================================================
        NG = 4
        NOC = 4
        OCW = D // NOC  # 512
        proc = [2 * c + h for h in range(LT) for c in range(NCORES)]
        with tc.tile_pool(name="mp2", bufs=1) as m2, \
             tc.tile_pool(name="mp2sc", bufs=2) as s6:
            wgu_sb = [m2.tile([P, 2 * OM], BF16, tag=f"wgu{k}", name=f"wgu{k}")
                      for k in range(KT)]
            # first two activation tiles before the weights, so the first
            # gate/up matmuls race the (3-queue) weight stream
            pre_a2t = {}
            a2t_src = [ag2c_out[h].rearrange(
                "(c k p) j -> c p k j", c=NCORES, k=KT, p=P)
                for h in range(LT)]
            for t in (0, 2):
                pa = s6.tile([P, D], BF16, tag="a2t", bufs=2)
                nc.sync.dma_start(pa[:], a2t_src[t % 2][t // 2])
                pre_a2t[t] = pa
            engs = [nc.sync, nc.scalar, nc.gpsimd]
            for k in range(KT):
                engs[k % 3].dma_start(wgu_sb[k][:], wguT[k * P:(k + 1) * P, :])

            def load_scales_half(h):
                hs = slice(h * 8, (h + 1) * 8)
                nc.scalar.dma_start(
                    as2g[:, hs],
                    ag2s_hout[h].rearrange("(c p) o -> p (c o)", p=P)
                )
                nc.vector.tensor_scalar(
                    sc_g[:, hs], as2g[:, hs], cb[3][:, 0:1], None,
                    op0=ALU.mult
                )
                nc.vector.tensor_scalar(
                    sc_u[:, hs], as2g[:, hs], cb[4][:, 0:1], None,
                    op0=ALU.mult
                )

            load_scales_half(0)
            # one-hot row-select (host input): picks this core's own tile
            # column out of the AR'd scale tiles -> local down-proj scale,
            # replacing v2's ReduceScatter(max)
            rsel = m2.tile([P, NCORES], F32, tag="rsel")
            rselr = s6.tile([1, NCORES], F32, tag="rselr", bufs=1)
            nc.scalar.dma_start(rselr[:], rowsel_d)
            nc.gpsimd.partition_broadcast(rsel[:], rselr[0:1, :])

            asm_in = [dp.tile([4 * P, 1], F32, tag=f"asmi{g}", name=f"asmi{g}")
                      for g in range(NG)]
            asm_go = [dp.tile([4 * P, 1], F32, tag=f"asmo{g}", name=f"asmo{g}",
                              addr_space="Shared") for g in range(NG)]
            mT = [m2.tile([P, R], BF16, tag=f"mT{kb}", name=f"mT{kb}")
                  for kb in range(MT)]
            m_tiles = [m2.tile([P, OM], BF16, tag=f"m{t}", name=f"m{t}")
                       for t in range(RT)]
            x1r = [m2.tile([P, D], F32, tag=f"x1r{lt}", name=f"x1r{lt}")
                   for lt in range(LT)]
            for lt in range(LT):
                nc.sync.dma_start(x1r[lt][:], x1_d[lt * P:(lt + 1) * P, :])
            rs_in = [dp.tile([NCORES * P, OCW], BF16, tag=f"rsdi{i}",
                             name=f"rsdi{i}") for i in range(NOC * LT)]
            rs_out = [dp.tile([P, OCW], BF16, tag=f"rsdo{i}",
                              name=f"rsdo{i}") for i in range(NOC * LT)]

            def gate_up_compute(g, ps6):
                for tl in range(4):
                    pi = g * 4 + tl
                    t = proc[pi]
                    if t in pre_a2t:
                        a2t = pre_a2t[t]
                    else:
                        a2t = s6.tile([P, D], BF16, tag="a2t", bufs=2)
                        nc.sync.dma_start(a2t[:], a2t_src[t % 2][t // 2])
                    psg = []
                    for n in range(4):
                        pg = ps6.tile([P, 512], F32, tag="psg", bufs=4)
                        for kb in range(KT):
                            nc.tensor.matmul(
                                pg[:], a2t[:, kb * P:(kb + 1) * P],
                                wgu_sb[kb][:, n * 512:(n + 1) * 512],
                                start=(kb == 0), stop=(kb == KT - 1),
                            )
                        psg.append(pg)
                    for half in range(2):
                        # silu(g_deq) = g_deq * sigmoid(g_deq)
                        sig = s6.tile([P, 512], F32, tag="sig")
                        nc.scalar.activation(
                            sig[:], psg[half][:], AF.Sigmoid,
                            scale=sc_g[:, pi:pi + 1]
                        )
                        sgl = s6.tile([P, 512], F32, tag="sgl")
                        nc.vector.scalar_tensor_tensor(
                            sgl[:], psg[half][:], sc_g[:, pi:pi + 1],
                            sig[:], op0=ALU.mult, op1=ALU.mult,
                        )
                        nc.vector.scalar_tensor_tensor(
                            m_tiles[t][:, half * 512:(half + 1) * 512],
                            psg[2 + half][:], sc_u[:, pi:pi + 1], sgl[:],
                            op0=ALU.mult, op1=ALU.mult,
                        )
                    nc.vector.tensor_reduce(
                        asm[:, pi:pi + 1], m_tiles[t][:], op=ALU.max,
                        axis=AX.X, apply_absolute_value=True,
                    )
                gs = slice(g * 4, (g + 1) * 4)
                nc.gpsimd.dma_start(
                    asm_in[g].rearrange("(t p) o -> p (t o)", p=P),
                    asm[:, gs],
                )
                nc.gpsimd.collective_compute(
                    "AllReduce", ALU.max, replica_groups=rg,
                    ins=[asm_in[g].opt()], outs=[asm_go[g].opt()],
                )

            def post_ar(g, ps6):
                # emitted >=1 group after the AR fire, so none of these
                # queue instructions ever waits on an in-flight collective
                gs = slice(g * 4, (g + 1) * 4)
                nc.scalar.dma_start(
                    asmg[:, gs],
                    asm_go[g].rearrange("(t p) o -> p (t o)", p=P),
                )
                nc.vector.tensor_scalar(
                    asmg[:, gs], asmg[:, gs], 1e-8, None, op0=ALU.add
                )
                nc.vector.reciprocal(qsm[:, gs], asmg[:, gs])
                nc.vector.tensor_scalar(
                    qsm[:, gs], qsm[:, gs], 127.0, None, op0=ALU.mult
                )
                for tl in range(4):
                    pi = g * 4 + tl
                    t = proc[pi]
                    m_q = s6.tile([P, OM], BF16, tag="m_q", bufs=2)
                    _quant(nc, s6, m_tiles[t][:], qsm[:, pi:pi + 1],
                           m_q[:], tag="qtmp")
                    # PE transposes: DMA transposes (1.2us each) would
                    # occupy the sync/scalar queues ~40us per group and
                    # starve the a2t/wd loads behind them
                    for kb in range(MT):
                        pst = ps6.tile([P, P], BF16, tag="pst6", bufs=2)
                        nc.tensor.transpose(
                            pst[:], m_q[:, kb * P:(kb + 1) * P], ident[:]
                        )
                        nc.vector.tensor_copy(
                            mT[kb][:, t * P:(t + 1) * P], pst[:]
                        )

            def sc_dl_half(lt):
                # sc_dl[:, lt] = asmg[pi of own tile 2c+lt] * wsd/127 via
                # one-hot mask (columns lt*8..lt*8+8 are all written by now)
                tmpm = s6.tile([P, NCORES], F32, tag="sctmp", bufs=1)
                nc.vector.tensor_tensor(
                    tmpm[:], asmg[:, lt * 8:(lt + 1) * 8], rsel[:],
                    op=ALU.mult
                )
                scr = s6.tile([P, 1], F32, tag="scr", bufs=1)
                nc.vector.tensor_reduce(
                    scr[:], tmpm[:], op=ALU.max, axis=AX.X
                )
                nc.vector.tensor_scalar(
                    sc_dl[:, lt:lt + 1], scr[:], cb[5][:, 0:1], None,
                    op0=ALU.mult
                )

            def down_pass(par, p7w, ps6, crange=range(NCORES), fire=True):
                for oc in range(NOC):
                    wd_oc = p7w.tile([P, MT * OCW], BF16, tag="wd_oc")
                    for kb in range(MT):
                        eng = nc.sync if kb % 2 == 0 else nc.gpsimd
                        eng.dma_start(
                            wd_oc[:, kb * OCW:(kb + 1) * OCW],
                            wdT[kb * P:(kb + 1) * P,
                                oc * OCW:(oc + 1) * OCW],
                        )
                    ci = oc * 2 + par
                    for c in crange:
                        t = 2 * c + par
                        pso = ps6.tile([P, OCW], F32, tag="dps", bufs=2)
                        for kb in range(MT):
                            nc.tensor.matmul(
                                pso[:], mT[kb][:, t * P:(t + 1) * P],
                                wd_oc[:, kb * OCW:(kb + 1) * OCW],
                                start=(kb == 0), stop=(kb == MT - 1),
                            )
                        # bf16 RS halves wire traffic; partials are ints
                        # < 2^18 so bf16 adds ~2^-9 relative rounding
                        dsb = s6.tile([P, OCW], BF16, tag="dsb", bufs=3)
                        nc.vector.tensor_copy(dsb[:], pso[:])
                        nc.sync.dma_start(
                            rs_in[ci][c * P:(c + 1) * P, :], dsb[:]
                        )
                    if fire:
                        nc.gpsimd.collective_compute(
                            "ReduceScatter", ALU.add, replica_groups=rg,
                            ins=[rs_in[ci].opt()], outs=[rs_out[ci].opt()],
                        )

            def down_drain(par):
                for oc in range(NOC):
                    ci = oc * 2 + par
                    ysb = s6.tile([P, OCW], BF16, tag="ysb")
                    nc.gpsimd.dma_start(ysb[:], rs_out[ci][:])
                    ot = s6.tile([P, OCW], F32, tag="ot")
                    nc.vector.scalar_tensor_tensor(
                        ot[:], ysb[:], sc_dl[:, par:par + 1],
                        x1r[par][:, oc * OCW:(oc + 1) * OCW],
                        op0=ALU.mult, op1=ALU.add,
                    )
                    nc.sync.dma_start(
                        out_d[par * P:(par + 1) * P,
                              oc * OCW:(oc + 1) * OCW],
                        ot[:],
                    )

            with tc.tile_pool(name="ps6", bufs=1, space="PSUM") as ps6, \
                 tc.tile_pool(name="p7w", bufs=2) as p7w:
                gate_up_compute(0, ps6)
                gate_up_compute(1, ps6)
                post_ar(0, ps6)
                load_scales_half(1)
                gate_up_compute(2, ps6)
                post_ar(1, ps6)
                sc_dl_half(0)
                # down-proj even row tiles: their mT strips (groups 0-1)
                # are transposed by now; the 4 RS-evens overlap group 3
                down_pass(0, p7w, ps6)
                post_ar(2, ps6)
                gate_up_compute(3, ps6)
                # down-proj odd tiles of groups 2 first: they are ready
                # before the last AllReduce lands
                down_pass(1, p7w, ps6, crange=range(0, 4), fire=False)
                post_ar(3, ps6)
                sc_dl_half(1)
                down_drain(0)
                down_pass(1, p7w, ps6, crange=range(4, NCORES), fire=True)
                down_drain(1)

    nc.compile()
    return nc


def _ternary_quant(w):
    """BitNet weight quant on host: ternary bf16 (exact) + f32 scale."""
    ws = float(np.abs(np.asarray(w, np.float64)).mean()) + 1e-8
    q = np.clip(np.round(np.asarray(w, np.float64) / ws), -1.0, 1.0)
    return q.astype(ml_dtypes.bfloat16), np.float32(ws)


def _prep_in_maps(inputs):
    x = np.asarray(inputs["x"], np.float32).reshape(R, D)
    wq, wsq = _ternary_quant(inputs["wq"])
    wk, wsk = _ternary_quant(inputs["wk"])
    wv, wsv = _ternary_quant(inputs["wv"])
    wo, wso = _ternary_quant(inputs["wo"])
    wg, wsg = _ternary_quant(inputs["wg"])
    wu, wsu = _ternary_quant(inputs["wu"])
    wd, wsd = _ternary_quant(inputs["wd"])
    n1 = np.asarray(inputs["norm1_w"], np.float32).reshape(1, D)
    n2 = np.asarray(inputs["norm2_w"], np.float32).reshape(1, D)

    ident = np.eye(P, dtype=ml_dtypes.bfloat16)
    iv, jv = np.mgrid[0:P, 0:P]
    # transposed causal block mask: key row u > query col v is masked
    causal_t = np.where(iv <= jv, 0.0, -1e30).astype(np.float32)
    wconsts = np.array([[
        wsq * wsk * INV_SQRT_HD / (127.0 * 127.0),
        wsv / 127.0, wso / 127.0, wsg / 127.0, wsu / 127.0, wsd / 127.0,
        0.0, 0.0,
    ]], np.float32)
    woT_full = np.ascontiguousarray(wo.T)

    in_maps = []
    for c in range(NCORES):
        qs = slice(c * OQ, (c + 1) * OQ)
        ms = slice(c * OM, (c + 1) * OM)
        # per-head-major qkv columns: q-h0, q-h1, k-h0, k-h1, v-h0, v-h1
        rowsel = np.zeros((1, NCORES), np.float32)
        rowsel[0, c] = 1.0
        in_maps.append({
            "x_rows": np.ascontiguousarray(x[c * RL:(c + 1) * RL]),
            "rowsel": rowsel,
            "wqkvT": np.ascontiguousarray(
                np.concatenate([wq[qs], wk[qs], wv[qs]], 0).T
            ),
            "woT": woT_full,
            "wguT": np.ascontiguousarray(
                np.concatenate([wg[ms], wu[ms]], 0).T
            ),
            "wdT": np.ascontiguousarray(wd[:, ms].T),
            "norm1_w": n1,
            "norm2_w": n2,
            "ident_b": ident,
            "causal_t": causal_t,
            "wconsts": wconsts,
        })
    return in_maps


def kernel(**inputs) -> np.ndarray:
    global _CACHED_NC
    if _CACHED_NC is None:
        _CACHED_NC = build_program()
    nc = _CACHED_NC
    in_maps = _prep_in_maps(inputs)
    res = run_bass_kernel_spmd(nc, in_maps, core_ids=list(range(NCORES)))
    out = np.concatenate([res.results[c]["out"] for c in range(NCORES)], 0)
    return out.reshape(B, S, D).astype(np.float32)
